# revision 18
# baseline (speedup 1.0000x reference)
"""CavAttention Trainium2 kernel (fused-DVE rewrite).

Computation (per spatial location (b,h,w), L=5 "cav" slots, 8 heads x 32 dim):
  qkv = x @ w_qkv ; att = softmax_j(mask * q_i.k_j / sqrt(d)) ; o = att @ v ; out = o @ w_out + b_out

Distribution: shard the H axis (48) across the 8 cores (6 each); weights replicated.

Per-core layout: locations (b,h,w) ride the 128 SBUF partitions; (l, head, d)
rides the free axis in bf16 (DVE 2x packed mode: 0.52 ns/elem vs 1.04 at 1x).
Measured DVE cost law: 0.52 ns/elem (2x) + ~150 ns/instruction, so the
attention core is emitted as ~14 big fused ops per 128-location tile instead
of ~52 small ones: one broadcast QK mul [p,i,j,(m d)], a 5-op pairwise d-tree,
ACT exp (interleaved with the AV stage of the previous tile to hide the ACT
round-trip), a multiplicative bf16 {1,0} mask, j-reduce, fast reciprocal,
softmax normalize, one 4-dim broadcast AV mul [p,i,j,d,m], and a 3-op j-tree.
The steady state is gapless on DVE (~14.5 us/tile, PE/ACT/DMA all inside it).

The output projection runs transposed: w_out chunks are the PE stationary and
the transposed attention output streams through, producing out^T (c-major) in
PSUM. That kills the per-i bias matmuls (b_out is added on the host) and the
f32 staging copies; out^T is cast to bf16 on ACT (halving output DMA traffic)
and the host transposes back / upcasts.
"""

import numpy as np

B, L, H, W, C = 2, 5, 48, 176, 256
HEADS, DIM_HEAD = 8, 32
INNER = HEADS * DIM_HEAD  # 256
SCALE = DIM_HEAD ** -0.5
NCORES = 8
HP = H // NCORES  # 6 h-planes per core
NBH = B * HP      # 12 (b,h) blocks per core
LOCS = NBH * W    # 2112 locations per core
PTILE = 128       # locations per tile
NTILES = (LOCS + PTILE - 1) // PTILE  # 17

_cached = {}


def _pieces(s, e):
    """Split flat loc range [s,e) into (p0, b, h, w0, w1) pieces within (b,h) blocks."""
    out = []
    cur = s
    while cur < e:
        bh = cur // W
        w0 = cur % W
        w1 = min(W, w0 + (e - cur))
        out.append((cur - s, bh // HP, bh % HP, w0, w1))
        cur += w1 - w0
    return out


def _build_bass():
    import concourse.bass as bass
    import concourse.bacc as bacc
    import concourse.tile as tile
    from concourse import mybir
    from concourse.masks import make_identity

    f32 = mybir.dt.float32
    bf16 = mybir.dt.bfloat16

    nc = bacc.Bacc()
    # x arrives pre-transposed and pre-cast on the host: [cc, c, b, h, l, w]
    xT_d = nc.dram_tensor("xT", [2, 128, B, HP, L, W], bf16, kind="ExternalInput")
    # mask arrives as a bf16 multiplicative mask (1 valid / 0 masked)
    mb_d = nc.dram_tensor("mbias", [B, HP, W, L], bf16, kind="ExternalInput")
    wqkv_d = nc.dram_tensor("w_qkv", [C, 3 * INNER], bf16, kind="ExternalInput")
    wout_d = nc.dram_tensor("w_out", [INNER, C], bf16, kind="ExternalInput")
    # out^T, bf16, pre-bias: element (o, c, b, h, l, w) = out[b, l, h, w, o*128+c]
    outT_d = nc.dram_tensor("outT", [2, 128, B, HP, L, W], bf16, kind="ExternalOutput")

    with tile.TileContext(nc) as tc:
        with (
            tc.tile_pool(name="singles", bufs=1) as singles,
            tc.tile_pool(name="work", bufs=3) as work,
            tc.tile_pool(name="peri", bufs=2) as peri,
            tc.tile_pool(name="ps_t", bufs=1, space="PSUM") as ps_t,
            tc.tile_pool(name="ps_qkv", bufs=2, space="PSUM") as ps_qkv,
            tc.tile_pool(name="ps_o", bufs=1, space="PSUM") as ps_o,
        ):
            # ---- constants.  Tensors touched by PE matmuls are produced by
            #      ONE engine (ACT): PE instructions carry a single
            #      semaphore wait (walrus S3_LW limit).
            #      Order: w_qkv DMA + cast first — it gates proj(0), the head
            #      of the pipeline-fill critical path. ----
            # clock warm-up: dummy DVE + PE work while the weight DMAs are in
            # flight, so the activity monitor upshifts the core clock before
            # the first real compute (early ops otherwise run ~60% slow)
            cw = singles.tile([128, 4096], bf16)
            nc.vector.memzero(cw[:, 0:4096])
            for _ in range(3):
                nc.vector.tensor_mul(cw[:, 0:2048], cw[:, 0:2048], cw[:, 2048:4096])
            ps_w = ps_o.tile([128, 512], f32, tag="pso", name="ps_w")
            for r in range(10):
                nc.tensor.matmul(
                    out=ps_w, lhsT=cw[:, 0:128], rhs=cw[:, 0:512],
                    start=True, stop=True, skip_group_check=True,
                )

            # weights arrive bf16 from the host (they are used in bf16 anyway);
            # staged through an ACT copy so PE matmul operands keep a single
            # producing engine (walrus single-wait limit)
            wqkv_l = singles.tile([128, 2, 3 * INNER], bf16)
            wqkv_sb = singles.tile([128, 2, 3 * INNER], bf16)
            for cc in range(2):
                # per-cc DMA + copy: proj(0)'s cc=0 matmuls start while the
                # cc=1 half is still in flight
                nc.sync.dma_start(out=wqkv_l[:, cc, :], in_=wqkv_d[cc * 128:(cc + 1) * 128, :])
                nc.scalar.copy(out=wqkv_sb[:, cc], in_=wqkv_l[:, cc])
            ident_l = singles.tile([128, 128], f32)
            make_identity(nc, ident_l)  # gpsimd
            wout_l = singles.tile([128, 2, C], bf16)
            nc.sync.dma_start(
                out=wout_l,
                in_=wout_d[:, :].rearrange("(cc p) n -> p cc n", cc=2),
            )

            ident = singles.tile([128, 128], bf16)
            wout_sb = singles.tile([128, 2, C], bf16)

            def setup_tail():
                """ident/wout casts — needed first by stage_b_out(0) in
                iteration 2; emitted after proj(0) so they don't sit ahead of
                the fill-critical qkv psum->sbuf copies in the ACT queue."""
                nc.scalar.copy(out=ident, in_=ident_l)
                nc.scalar.copy(out=wout_sb, in_=wout_l)

            def load(t):
                """DMA in (x already transposed+bf16 on host; partitions = C-chunk)."""
                s = t * PTILE
                e = min(s + PTILE, LOCS)
                P = e - s
                pieces = _pieces(s, e)

                xt = work.tile([128, 2, L, 128], bf16, tag="xt")
                for (p0, b, h, w0, w1) in pieces:
                    for cc in range(2):
                        nc.sync.dma_start(
                            out=xt[:, cc, :, p0:p0 + (w1 - w0)],
                            in_=xT_d[cc, :, b, h, :, w0:w1],
                        )
                mbias = work.tile([128, L], bf16, tag="mbias")
                for (p0, b, h, w0, w1) in pieces:
                    nc.sync.dma_start(
                        out=mbias[p0:p0 + (w1 - w0), :],
                        in_=mb_d[b, h, w0:w1, :],
                    )
                return dict(P=P, pieces=pieces, xt=xt, mbias=mbias)

            def proj(st):
                """qkv projection on PE + ACT psum->sbuf cast."""
                P = st["P"]
                xt = st["xt"]
                qkv_bf = work.tile([128, L, 3 * INNER], bf16, tag="qkv_bf")
                for l in range(L):
                    pq = ps_qkv.tile([128, 3 * INNER], f32, tag="psq")
                    for cc in range(2):
                        for (n0, n1) in ((0, 512), (512, 768)):
                            nc.tensor.matmul(
                                out=pq[:P, n0:n1],
                                lhsT=xt[:, cc, l, :P],
                                rhs=wqkv_sb[:, cc, n0:n1],
                                start=(cc == 0),
                                stop=(cc == 1),
                            )
                    nc.scalar.copy(out=qkv_bf[:P, l, :], in_=pq[:P, :])
                st["qkv_bf"] = qkv_bf

            def stage_a1(st):
                """QK^T + mask bias on DVE; kicks off ACT exp.  -> st['am' .. ]"""
                P = st["P"]
                qkv_bf = st["qkv_bf"]
                # q[p, i, (m d)] bcast over j;  k[p, j, (m d)] bcast over i
                q_v = qkv_bf[:P, :, 0:INNER].unsqueeze(2).broadcast_to([P, L, L, INNER])
                k_v = qkv_bf[:P, :, INNER:2 * INNER].unsqueeze(1).broadcast_to([P, L, L, INNER])
                qk = peri.tile([128, L * L * HEADS, DIM_HEAD], bf16, tag="qk")
                nc.vector.tensor_mul(
                    qk[:P].rearrange("p (i j m) d -> p i j (m d)", i=L, j=L),
                    q_v, k_v,
                )
                # pairwise d-tree: 32 -> 16 -> 8 -> 4 -> 2 -> 1
                t16 = peri.tile([128, L * L * HEADS, 16], bf16, tag="t16")
                nc.vector.tensor_add(t16[:P], qk[:P, :, 0:16], qk[:P, :, 16:32])
                t8 = peri.tile([128, L * L * HEADS, 8], bf16, tag="t8")
                nc.vector.tensor_add(t8[:P], t16[:P, :, 0:8], t16[:P, :, 8:16])
                t4 = peri.tile([128, L * L * HEADS, 4], bf16, tag="t4")
                nc.vector.tensor_add(t4[:P], t8[:P, :, 0:4], t8[:P, :, 4:8])
                t2 = peri.tile([128, L * L * HEADS, 2], bf16, tag="t2")
                nc.vector.tensor_add(t2[:P], t4[:P, :, 0:2], t4[:P, :, 2:4])
                # final fold in f32 (exp-input precision)
                A = peri.tile([128, L, L, HEADS], f32, tag="A")
                nc.vector.tensor_add(
                    A[:P],
                    t2[:P, :, 0].rearrange("p (i j m) -> p i j m", i=L, j=L),
                    t2[:P, :, 1].rearrange("p (i j m) -> p i j m", i=L, j=L),
                )
                # exp on ACT (runs while DVE does the AV stage of tile t-2)
                eu = peri.tile([128, L, L, HEADS], bf16, tag="eu")
                nc.scalar.activation(
                    out=eu[:P], in_=A[:P], func=mybir.ActivationFunctionType.Exp,
                    scale=SCALE,
                )
                st["eu"] = eu

            def stage_a2(st):
                """softmax tail: mask, j-sum, reciprocal, normalize -> st['pw']."""
                P = st["P"]
                # multiplicative {1,0} mask after exp (bf16 2x beats the f32
                # additive -1e4 logit bias, and halves the mask DMA); placed
                # here so the AV stage of tile t-2 hides the exp round-trip
                ee = work.tile([128, L, L, HEADS], bf16, tag="ee")
                nc.vector.tensor_mul(
                    ee[:P], st["eu"][:P],
                    st["mbias"][:P].unsqueeze(1).unsqueeze(3).broadcast_to([P, L, L, HEADS]),
                )
                ssum = work.tile([128, L, HEADS], f32, tag="ssum")
                nc.vector.reduce_sum(
                    out=ssum[:P], in_=ee[:P].transpose([0, 1, 3, 2]), axis=mybir.AxisListType.X
                )
                # ~51 ULP approx, ~5x faster than iterative reciprocal; ssum is
                # strictly positive and well inside the safe range
                sinv = work.tile([128, L, HEADS], f32, tag="sinv")
                nc.vector.reciprocal_approx_fast(out=sinv[:P], in_=ssum[:P])
                pw = work.tile([128, L, L, HEADS], bf16, tag="pw")
                nc.vector.tensor_mul(
                    pw[:P], ee[:P],
                    sinv[:P].unsqueeze(2).broadcast_to([P, L, L, HEADS]),
                )
                st["pw"] = pw

            def stage_b_av(st, ia, ib):
                """attention-weighted V for i in [ia, ib): one 4-dim broadcast
                mul + 3-op j-tree.

                V rides in (d, m) order (host-permuted w_qkv columns) so the pw
                broadcast lands on a non-inner dim; w_out rows are host-permuted
                to match."""
                P = st["P"]
                ni = ib - ia
                pw = st["pw"]
                qkv_bf = st["qkv_bf"]
                # v[p, j, d, m] bcast over i;  pw[p, i, j, m] bcast over d
                v4 = (qkv_bf[:P, :, 2 * INNER:3 * INNER]
                      .rearrange("p j (d m) -> p j d m", m=HEADS)
                      .unsqueeze(1).broadcast_to([P, ni, L, DIM_HEAD, HEADS]))
                pw4 = pw[:P, ia:ib].unsqueeze(3).broadcast_to([P, ni, L, DIM_HEAD, HEADS])
                if "av" not in st:
                    st["av"] = peri.tile([128, L, L, INNER], bf16, tag="av", name="av")
                    st["s2"] = peri.tile([128, L, 2, INNER], bf16, tag="s2", name="s2")
                    st["s1"] = peri.tile([128, L, INNER], bf16, tag="s1", name="s1")
                    st["attout"] = work.tile([128, L, INNER], bf16, tag="attout", name="attout")
                av, s2, s1, attout = st["av"], st["s2"], st["s1"], st["attout"]
                nc.vector.tensor_mul(
                    av[:P, ia:ib].rearrange("p i j (d m) -> p i j d m", m=HEADS), v4, pw4,
                )
                # j-tree: 5 -> (2+2) -> 1 (+ leftover j=4)
                nc.vector.tensor_add(s2[:P, ia:ib], av[:P, ia:ib, 0:2], av[:P, ia:ib, 2:4])
                nc.vector.tensor_add(s1[:P, ia:ib], s2[:P, ia:ib, 0], s2[:P, ia:ib, 1])
                nc.vector.tensor_add(attout[:P, ia:ib], s1[:P, ia:ib], av[:P, ia:ib, 4])

            def stage_b_out(st, ia, ib):
                """PE transposes + transposed out-projection + bf16 store, for
                i in [ia, ib)."""
                P = st["P"]
                attout = st["attout"]
                # transpose attout[:, i, cc*128:(cc+1)*128] -> pt[:, i, cc, :P]
                if "pt" not in st:
                    st["pt"] = ps_t.tile([128, L, 2, 128], bf16, tag="pst", name="pt")
                    st["aoti"] = peri.tile([128, L, 2, 128], bf16, tag="aoti", name="aoti")
                    st["osb"] = peri.tile([128, 2, L, 128], bf16, tag="osb", name="osb")
                pt, aoti, osb = st["pt"], st["aoti"], st["osb"]
                for i in range(ia, ib):
                    for cc in range(2):
                        nc.tensor.transpose(
                            pt[:, i, cc, :P],
                            attout[:P, i, cc * 128:(cc + 1) * 128],
                            ident[:P, :P],
                        )
                nc.scalar.copy(out=aoti[:, ia:ib], in_=pt[:, ia:ib])
                # out^T[c_chunk, (i, loc)] = sum_cc wout[cc, c_chunk]^T @ aoti[cc]
                # accumulation groups must stay inside one 2KB PSUM bank
                # (f32 col 512 == i 4), so split i-ranges at 4.
                igroups = [(a, b) for (a, b) in ((ia, min(ib, 4)), (max(ia, 4), ib)) if a < b]
                for o in range(2):
                    po = ps_o.tile([128, L, 128], f32, tag="pso")
                    for (i0, i1) in igroups:
                        for cc in range(2):
                            nc.tensor.matmul(
                                out=po[:, i0:i1, :P],
                                lhsT=wout_sb[:, cc, o * 128:(o + 1) * 128],
                                rhs=aoti[:, i0:i1, cc, :P],
                                start=(cc == 0),
                                stop=(cc == 1),
                            )
                    nc.scalar.copy(out=osb[:, o, ia:ib], in_=po[:, ia:ib])
                for (p0, b, h, w0, w1) in st["pieces"]:
                    for o in range(2):
                        nc.sync.dma_start(
                            out=outT_d[o, :, b, h, ia:ib, w0:w1],
                            in_=osb[:, o, ia:ib, p0:p0 + (w1 - w0)],
                        )

            # 3-deep software pipeline: per iteration t issue
            #   load(t)+proj(t)    DMA + PE qkv + ACT cast
            #   stage_a1(t-1)      DVE qk -> ACT exp
            #   stage_b(t-2)       DVE av (covers the exp round-trip)
            #   stage_a2(t-1)      DVE softmax tail
            #   stage_b_out(t-2)   PE transpose + out-proj; ACT cast; DMA store
            sts = {}
            for t in range(NTILES + 2):
                if t < NTILES:
                    sts[t] = load(t)
                    proj(sts[t])
                    if t == 0:
                        setup_tail()
                if 0 <= t - 1 < NTILES:
                    stage_a1(sts[t - 1])
                if 0 <= t - 2 < NTILES:
                    if t - 2 == NTILES - 1:
                        # last tile: chunk the backend per-i so its PE/ACT/DMA
                        # tail overlaps the tail of the DVE work (pipeline drain)
                        for i in range(L):
                            stage_b_av(sts[t - 2], i, i + 1)
                            stage_b_out(sts[t - 2], i, i + 1)
                    else:
                        stage_b_av(sts[t - 2], 0, 5)
                if 0 <= t - 1 < NTILES:
                    stage_a2(sts[t - 1])
                if 0 <= t - 2 < NTILES:
                    if t - 2 != NTILES - 1:
                        stage_b_out(sts[t - 2], 0, 5)
                    del sts[t - 2]
    nc.finalize()  # Bacc.compile(): legalize multi-wait instructions, alloc regs
    return nc


def get_nc():
    if "nc" not in _cached:
        _cached["nc"] = _build_bass()
    return _cached["nc"]


def make_in_maps(x, mask, w_qkv, w_out, b_out):
    """Host-side shard + repack: x is transposed to [cc, c, b, h, l, w] and
    cast to bf16; the mask becomes an f32 additive logit bias."""
    import ml_dtypes

    x = np.asarray(x, dtype=np.float32)
    mask = np.asarray(mask)
    w_qkv = np.ascontiguousarray(np.asarray(w_qkv), dtype=np.float32)
    w_out = np.ascontiguousarray(np.asarray(w_out), dtype=np.float32)

    # permute V's output columns (m,d)->(d,m) and w_out's rows to match, so
    # the device-side pw broadcast is never on the innermost dim
    wv = w_qkv[:, 2 * INNER:].reshape(C, HEADS, DIM_HEAD).transpose(0, 2, 1).reshape(C, INNER)
    w_qkv = np.ascontiguousarray(
        np.concatenate([w_qkv[:, :2 * INNER], wv], axis=1).astype(ml_dtypes.bfloat16)
    )
    w_out = np.ascontiguousarray(
        w_out.reshape(HEADS, DIM_HEAD, C).transpose(1, 0, 2).reshape(INNER, C)
        .astype(ml_dtypes.bfloat16)
    )

    # [B, L, H, W, C] -> [C, B, H, L, W] -> [2, 128, B, H, L, W] bf16
    xT = np.transpose(x, (4, 0, 2, 1, 3)).astype(ml_dtypes.bfloat16)
    xT = np.ascontiguousarray(xT.reshape(2, 128, B, H, L, W))
    # [B, H, W, 1, L] -> bf16 multiplicative mask [B, H, W, L]
    mb = np.ascontiguousarray(
        (mask[:, :, :, 0, :] != 0).astype(ml_dtypes.bfloat16)
    )

    in_maps = []
    for k in range(NCORES):
        h0, h1 = k * HP, (k + 1) * HP
        in_maps.append({
            "xT": np.ascontiguousarray(xT[:, :, :, h0:h1]),
            "mbias": np.ascontiguousarray(mb[:, h0:h1]),
            "w_qkv": w_qkv,
            "w_out": w_out,
        })
    return in_maps


def assemble_out(results, b_out):
    """Host-side unshard: out^T bf16 [2, 128, B, HP, L, W] per core ->
    full f32 [B, L, H, W, C] (+ b_out)."""
    outT = np.concatenate([r["outT"] for r in results], axis=3)  # [2,128,B,H,L,W]
    out = np.transpose(outT, (2, 4, 3, 5, 0, 1)).reshape(B, L, H, W, C)
    return out.astype(np.float32) + np.asarray(b_out, dtype=np.float32)


def kernel(x, mask, w_qkv, w_out, b_out):
    from concourse.bass_utils import run_bass_kernel_spmd

    nc = get_nc()
    in_maps = make_in_maps(x, mask, w_qkv, w_out, b_out)
    res = run_bass_kernel_spmd(nc, in_maps, core_ids=list(range(NCORES)))
    return assemble_out(res.results, b_out)


# revision 19
# speedup vs baseline: 1.0060x; 1.0060x over previous
"""CavAttention Trainium2 kernel (fused-DVE rewrite).

Computation (per spatial location (b,h,w), L=5 "cav" slots, 8 heads x 32 dim):
  qkv = x @ w_qkv ; att = softmax_j(mask * q_i.k_j / sqrt(d)) ; o = att @ v ; out = o @ w_out + b_out

Distribution: shard the H axis (48) across the 8 cores (6 each); weights replicated.

Per-core layout: locations (b,h,w) ride the 128 SBUF partitions; (l, head, d)
rides the free axis in bf16 (DVE 2x packed mode: 0.52 ns/elem vs 1.04 at 1x).
Measured DVE cost law: 0.52 ns/elem (2x) + ~150 ns/instruction, so the
attention core is emitted as ~14 big fused ops per 128-location tile instead
of ~52 small ones: one broadcast QK mul [p,i,j,(m d)], a 5-op pairwise d-tree,
ACT exp (interleaved with the AV stage of the previous tile to hide the ACT
round-trip), a multiplicative bf16 {1,0} mask, j-reduce, fast reciprocal,
softmax normalize, one 4-dim broadcast AV mul [p,i,j,d,m], and a 3-op j-tree.
The steady state is gapless on DVE (~14.5 us/tile, PE/ACT/DMA all inside it).

The output projection runs transposed: w_out chunks are the PE stationary and
the transposed attention output streams through, producing out^T (c-major) in
PSUM. That kills the per-i bias matmuls (b_out is added on the host) and the
f32 staging copies; out^T is cast to bf16 on ACT (halving output DMA traffic)
and the host transposes back / upcasts.
"""

import numpy as np

B, L, H, W, C = 2, 5, 48, 176, 256
HEADS, DIM_HEAD = 8, 32
INNER = HEADS * DIM_HEAD  # 256
SCALE = DIM_HEAD ** -0.5
NCORES = 8
HP = H // NCORES  # 6 h-planes per core
NBH = B * HP      # 12 (b,h) blocks per core
LOCS = NBH * W    # 2112 locations per core
PTILE = 128       # locations per tile
NTILES = (LOCS + PTILE - 1) // PTILE  # 17

_cached = {}


def _pieces(s, e):
    """Split flat loc range [s,e) into (p0, b, h, w0, w1) pieces within (b,h) blocks."""
    out = []
    cur = s
    while cur < e:
        bh = cur // W
        w0 = cur % W
        w1 = min(W, w0 + (e - cur))
        out.append((cur - s, bh // HP, bh % HP, w0, w1))
        cur += w1 - w0
    return out


def _build_bass():
    import concourse.bass as bass
    import concourse.bacc as bacc
    import concourse.tile as tile
    from concourse import mybir
    from concourse.masks import make_identity

    f32 = mybir.dt.float32
    bf16 = mybir.dt.bfloat16

    nc = bacc.Bacc()
    # x arrives pre-transposed and pre-cast on the host: [cc, c, b, h, l, w]
    xT_d = nc.dram_tensor("xT", [2, 128, B, HP, L, W], bf16, kind="ExternalInput")
    # mask arrives as a bf16 multiplicative mask (1 valid / 0 masked)
    mb_d = nc.dram_tensor("mbias", [B, HP, W, L], bf16, kind="ExternalInput")
    wqkv_d = nc.dram_tensor("w_qkv", [C, 3 * INNER], bf16, kind="ExternalInput")
    wout_d = nc.dram_tensor("w_out", [INNER, C], bf16, kind="ExternalInput")
    # out^T, bf16, pre-bias: element (o, c, b, h, l, w) = out[b, l, h, w, o*128+c]
    outT_d = nc.dram_tensor("outT", [2, 128, B, HP, L, W], bf16, kind="ExternalOutput")

    with tile.TileContext(nc) as tc:
        with (
            tc.tile_pool(name="singles", bufs=1) as singles,
            tc.tile_pool(name="work", bufs=3) as work,
            tc.tile_pool(name="peri", bufs=2) as peri,
            tc.tile_pool(name="ps_t", bufs=1, space="PSUM") as ps_t,
            tc.tile_pool(name="ps_qkv", bufs=2, space="PSUM") as ps_qkv,
            tc.tile_pool(name="ps_o", bufs=1, space="PSUM") as ps_o,
        ):
            # ---- constants.  Tensors touched by PE matmuls are produced by
            #      ONE engine (ACT): PE instructions carry a single
            #      semaphore wait (walrus S3_LW limit).
            #      Order: w_qkv DMA + cast first — it gates proj(0), the head
            #      of the pipeline-fill critical path. ----
            # clock warm-up: dummy DVE + PE work while the weight DMAs are in
            # flight, so the activity monitor upshifts the core clock before
            # the first real compute (early ops otherwise run ~60% slow)
            cw = singles.tile([128, 4096], bf16)
            nc.vector.memzero(cw[:, 0:4096])
            for _ in range(3):
                nc.vector.tensor_mul(cw[:, 0:2048], cw[:, 0:2048], cw[:, 2048:4096])
            # weights arrive bf16 from the host (they are used in bf16 anyway);
            # staged through an ACT copy so PE matmul operands keep a single
            # producing engine (walrus single-wait limit)
            wqkv_l = singles.tile([128, 2, 3 * INNER], bf16)
            wqkv_sb = singles.tile([128, 2, 3 * INNER], bf16)
            for cc in range(2):
                # per-cc DMA + copy: proj(0)'s cc=0 matmuls start while the
                # cc=1 half is still in flight
                nc.sync.dma_start(out=wqkv_l[:, cc, :], in_=wqkv_d[cc * 128:(cc + 1) * 128, :])
                nc.scalar.copy(out=wqkv_sb[:, cc], in_=wqkv_l[:, cc])
            ident_l = singles.tile([128, 128], f32)
            make_identity(nc, ident_l)  # gpsimd
            wout_l = singles.tile([128, 2, C], bf16)
            nc.sync.dma_start(
                out=wout_l,
                in_=wout_d[:, :].rearrange("(cc p) n -> p cc n", cc=2),
            )

            ident = singles.tile([128, 128], bf16)
            wout_sb = singles.tile([128, 2, C], bf16)

            def setup_tail():
                """ident/wout casts — needed first by stage_b_out(0) in
                iteration 2; emitted after proj(0) so they don't sit ahead of
                the fill-critical qkv psum->sbuf copies in the ACT queue."""
                nc.scalar.copy(out=ident, in_=ident_l)
                nc.scalar.copy(out=wout_sb, in_=wout_l)

            def load(t):
                """DMA in (x already transposed+bf16 on host; partitions = C-chunk)."""
                s = t * PTILE
                e = min(s + PTILE, LOCS)
                P = e - s
                pieces = _pieces(s, e)

                xt = work.tile([128, 2, L, 128], bf16, tag="xt")
                for (p0, b, h, w0, w1) in pieces:
                    for cc in range(2):
                        nc.sync.dma_start(
                            out=xt[:, cc, :, p0:p0 + (w1 - w0)],
                            in_=xT_d[cc, :, b, h, :, w0:w1],
                        )
                mbias = work.tile([128, L], bf16, tag="mbias")
                for (p0, b, h, w0, w1) in pieces:
                    nc.sync.dma_start(
                        out=mbias[p0:p0 + (w1 - w0), :],
                        in_=mb_d[b, h, w0:w1, :],
                    )
                return dict(P=P, pieces=pieces, xt=xt, mbias=mbias)

            def proj(st):
                """qkv projection on PE + ACT psum->sbuf cast."""
                P = st["P"]
                xt = st["xt"]
                qkv_bf = work.tile([128, L, 3 * INNER], bf16, tag="qkv_bf")
                for l in range(L):
                    pq = ps_qkv.tile([128, 3 * INNER], f32, tag="psq")
                    for cc in range(2):
                        for (n0, n1) in ((0, 512), (512, 768)):
                            nc.tensor.matmul(
                                out=pq[:P, n0:n1],
                                lhsT=xt[:, cc, l, :P],
                                rhs=wqkv_sb[:, cc, n0:n1],
                                start=(cc == 0),
                                stop=(cc == 1),
                            )
                    nc.scalar.copy(out=qkv_bf[:P, l, :], in_=pq[:P, :])
                st["qkv_bf"] = qkv_bf

            def stage_a1(st):
                """QK^T + mask bias on DVE; kicks off ACT exp.  -> st['am' .. ]"""
                P = st["P"]
                qkv_bf = st["qkv_bf"]
                # q[p, i, (m d)] bcast over j;  k[p, j, (m d)] bcast over i
                q_v = qkv_bf[:P, :, 0:INNER].unsqueeze(2).broadcast_to([P, L, L, INNER])
                k_v = qkv_bf[:P, :, INNER:2 * INNER].unsqueeze(1).broadcast_to([P, L, L, INNER])
                qk = peri.tile([128, L * L * HEADS, DIM_HEAD], bf16, tag="qk")
                nc.vector.tensor_mul(
                    qk[:P].rearrange("p (i j m) d -> p i j (m d)", i=L, j=L),
                    q_v, k_v,
                )
                # pairwise d-tree: 32 -> 16 -> 8 -> 4 -> 2 -> 1
                t16 = peri.tile([128, L * L * HEADS, 16], bf16, tag="t16")
                nc.vector.tensor_add(t16[:P], qk[:P, :, 0:16], qk[:P, :, 16:32])
                t8 = peri.tile([128, L * L * HEADS, 8], bf16, tag="t8")
                nc.vector.tensor_add(t8[:P], t16[:P, :, 0:8], t16[:P, :, 8:16])
                t4 = peri.tile([128, L * L * HEADS, 4], bf16, tag="t4")
                nc.vector.tensor_add(t4[:P], t8[:P, :, 0:4], t8[:P, :, 4:8])
                t2 = peri.tile([128, L * L * HEADS, 2], bf16, tag="t2")
                nc.vector.tensor_add(t2[:P], t4[:P, :, 0:2], t4[:P, :, 2:4])
                # final fold in f32 (exp-input precision)
                A = peri.tile([128, L, L, HEADS], f32, tag="A")
                nc.vector.tensor_add(
                    A[:P],
                    t2[:P, :, 0].rearrange("p (i j m) -> p i j m", i=L, j=L),
                    t2[:P, :, 1].rearrange("p (i j m) -> p i j m", i=L, j=L),
                )
                # exp on ACT (runs while DVE does the AV stage of tile t-2)
                eu = peri.tile([128, L, L, HEADS], bf16, tag="eu")
                nc.scalar.activation(
                    out=eu[:P], in_=A[:P], func=mybir.ActivationFunctionType.Exp,
                    scale=SCALE,
                )
                st["eu"] = eu

            def stage_a2(st):
                """softmax tail: mask, j-sum, reciprocal, normalize -> st['pw']."""
                P = st["P"]
                # multiplicative {1,0} mask after exp (bf16 2x beats the f32
                # additive -1e4 logit bias, and halves the mask DMA); placed
                # here so the AV stage of tile t-2 hides the exp round-trip
                ee = work.tile([128, L, L, HEADS], bf16, tag="ee")
                nc.vector.tensor_mul(
                    ee[:P], st["eu"][:P],
                    st["mbias"][:P].unsqueeze(1).unsqueeze(3).broadcast_to([P, L, L, HEADS]),
                )
                ssum = work.tile([128, L, HEADS], f32, tag="ssum")
                nc.vector.reduce_sum(
                    out=ssum[:P], in_=ee[:P].transpose([0, 1, 3, 2]), axis=mybir.AxisListType.X
                )
                # ~51 ULP approx, ~5x faster than iterative reciprocal; ssum is
                # strictly positive and well inside the safe range
                sinv = work.tile([128, L, HEADS], f32, tag="sinv")
                nc.vector.reciprocal_approx_fast(out=sinv[:P], in_=ssum[:P])
                pw = work.tile([128, L, L, HEADS], bf16, tag="pw")
                nc.vector.tensor_mul(
                    pw[:P], ee[:P],
                    sinv[:P].unsqueeze(2).broadcast_to([P, L, L, HEADS]),
                )
                st["pw"] = pw

            def stage_b_av(st, ia, ib):
                """attention-weighted V for i in [ia, ib): one 4-dim broadcast
                mul + 3-op j-tree.

                V rides in (d, m) order (host-permuted w_qkv columns) so the pw
                broadcast lands on a non-inner dim; w_out rows are host-permuted
                to match."""
                P = st["P"]
                ni = ib - ia
                pw = st["pw"]
                qkv_bf = st["qkv_bf"]
                # v[p, j, d, m] bcast over i;  pw[p, i, j, m] bcast over d
                v4 = (qkv_bf[:P, :, 2 * INNER:3 * INNER]
                      .rearrange("p j (d m) -> p j d m", m=HEADS)
                      .unsqueeze(1).broadcast_to([P, ni, L, DIM_HEAD, HEADS]))
                pw4 = pw[:P, ia:ib].unsqueeze(3).broadcast_to([P, ni, L, DIM_HEAD, HEADS])
                if "av" not in st:
                    st["av"] = peri.tile([128, L, L, INNER], bf16, tag="av", name="av")
                    st["s2"] = peri.tile([128, L, 2, INNER], bf16, tag="s2", name="s2")
                    st["s1"] = peri.tile([128, L, INNER], bf16, tag="s1", name="s1")
                    st["attout"] = work.tile([128, L, INNER], bf16, tag="attout", name="attout")
                av, s2, s1, attout = st["av"], st["s2"], st["s1"], st["attout"]
                nc.vector.tensor_mul(
                    av[:P, ia:ib].rearrange("p i j (d m) -> p i j d m", m=HEADS), v4, pw4,
                )
                # j-tree: 5 -> (2+2) -> 1 (+ leftover j=4)
                nc.vector.tensor_add(s2[:P, ia:ib], av[:P, ia:ib, 0:2], av[:P, ia:ib, 2:4])
                nc.vector.tensor_add(s1[:P, ia:ib], s2[:P, ia:ib, 0], s2[:P, ia:ib, 1])
                nc.vector.tensor_add(attout[:P, ia:ib], s1[:P, ia:ib], av[:P, ia:ib, 4])

            def stage_b_out(st, ia, ib):
                """PE transposes + transposed out-projection + bf16 store, for
                i in [ia, ib)."""
                P = st["P"]
                attout = st["attout"]
                # transpose attout[:, i, cc*128:(cc+1)*128] -> pt[:, i, cc, :P]
                if "pt" not in st:
                    st["pt"] = ps_t.tile([128, L, 2, 128], bf16, tag="pst", name="pt")
                    st["aoti"] = peri.tile([128, L, 2, 128], bf16, tag="aoti", name="aoti")
                    st["osb"] = peri.tile([128, 2, L, 128], bf16, tag="osb", name="osb")
                pt, aoti, osb = st["pt"], st["aoti"], st["osb"]
                for i in range(ia, ib):
                    for cc in range(2):
                        nc.tensor.transpose(
                            pt[:, i, cc, :P],
                            attout[:P, i, cc * 128:(cc + 1) * 128],
                            ident[:P, :P],
                        )
                nc.scalar.copy(out=aoti[:, ia:ib], in_=pt[:, ia:ib])
                # out^T[c_chunk, (i, loc)] = sum_cc wout[cc, c_chunk]^T @ aoti[cc]
                # accumulation groups must stay inside one 2KB PSUM bank
                # (f32 col 512 == i 4), so split i-ranges at 4.
                igroups = [(a, b) for (a, b) in ((ia, min(ib, 4)), (max(ia, 4), ib)) if a < b]
                for o in range(2):
                    po = ps_o.tile([128, L, 128], f32, tag="pso")
                    for (i0, i1) in igroups:
                        for cc in range(2):
                            nc.tensor.matmul(
                                out=po[:, i0:i1, :P],
                                lhsT=wout_sb[:, cc, o * 128:(o + 1) * 128],
                                rhs=aoti[:, i0:i1, cc, :P],
                                start=(cc == 0),
                                stop=(cc == 1),
                            )
                    nc.scalar.copy(out=osb[:, o, ia:ib], in_=po[:, ia:ib])
                for (p0, b, h, w0, w1) in st["pieces"]:
                    for o in range(2):
                        nc.sync.dma_start(
                            out=outT_d[o, :, b, h, ia:ib, w0:w1],
                            in_=osb[:, o, ia:ib, p0:p0 + (w1 - w0)],
                        )

            # 3-deep software pipeline: per iteration t issue
            #   load(t)+proj(t)    DMA + PE qkv + ACT cast
            #   stage_a1(t-1)      DVE qk -> ACT exp
            #   stage_b(t-2)       DVE av (covers the exp round-trip)
            #   stage_a2(t-1)      DVE softmax tail
            #   stage_b_out(t-2)   PE transpose + out-proj; ACT cast; DMA store
            sts = {}
            for t in range(NTILES + 2):
                if t < NTILES:
                    sts[t] = load(t)
                    proj(sts[t])
                    if t == 0:
                        setup_tail()
                if 0 <= t - 1 < NTILES:
                    stage_a1(sts[t - 1])
                if 0 <= t - 2 < NTILES:
                    if t - 2 == NTILES - 1:
                        # last tile: chunk the backend per-i so its PE/ACT/DMA
                        # tail overlaps the tail of the DVE work (pipeline drain)
                        for i in range(L):
                            stage_b_av(sts[t - 2], i, i + 1)
                            stage_b_out(sts[t - 2], i, i + 1)
                    else:
                        stage_b_av(sts[t - 2], 0, 5)
                if 0 <= t - 1 < NTILES:
                    stage_a2(sts[t - 1])
                if 0 <= t - 2 < NTILES:
                    if t - 2 != NTILES - 1:
                        stage_b_out(sts[t - 2], 0, 5)
                    del sts[t - 2]
    nc.finalize()  # Bacc.compile(): legalize multi-wait instructions, alloc regs
    return nc


def get_nc():
    if "nc" not in _cached:
        _cached["nc"] = _build_bass()
    return _cached["nc"]


def make_in_maps(x, mask, w_qkv, w_out, b_out):
    """Host-side shard + repack: x is transposed to [cc, c, b, h, l, w] and
    cast to bf16; the mask becomes an f32 additive logit bias."""
    import ml_dtypes

    x = np.asarray(x, dtype=np.float32)
    mask = np.asarray(mask)
    w_qkv = np.ascontiguousarray(np.asarray(w_qkv), dtype=np.float32)
    w_out = np.ascontiguousarray(np.asarray(w_out), dtype=np.float32)

    # permute V's output columns (m,d)->(d,m) and w_out's rows to match, so
    # the device-side pw broadcast is never on the innermost dim
    wv = w_qkv[:, 2 * INNER:].reshape(C, HEADS, DIM_HEAD).transpose(0, 2, 1).reshape(C, INNER)
    w_qkv = np.ascontiguousarray(
        np.concatenate([w_qkv[:, :2 * INNER], wv], axis=1).astype(ml_dtypes.bfloat16)
    )
    w_out = np.ascontiguousarray(
        w_out.reshape(HEADS, DIM_HEAD, C).transpose(1, 0, 2).reshape(INNER, C)
        .astype(ml_dtypes.bfloat16)
    )

    # [B, L, H, W, C] -> [C, B, H, L, W] -> [2, 128, B, H, L, W] bf16
    xT = np.transpose(x, (4, 0, 2, 1, 3)).astype(ml_dtypes.bfloat16)
    xT = np.ascontiguousarray(xT.reshape(2, 128, B, H, L, W))
    # [B, H, W, 1, L] -> bf16 multiplicative mask [B, H, W, L]
    mb = np.ascontiguousarray(
        (mask[:, :, :, 0, :] != 0).astype(ml_dtypes.bfloat16)
    )

    in_maps = []
    for k in range(NCORES):
        h0, h1 = k * HP, (k + 1) * HP
        in_maps.append({
            "xT": np.ascontiguousarray(xT[:, :, :, h0:h1]),
            "mbias": np.ascontiguousarray(mb[:, h0:h1]),
            "w_qkv": w_qkv,
            "w_out": w_out,
        })
    return in_maps


def assemble_out(results, b_out):
    """Host-side unshard: out^T bf16 [2, 128, B, HP, L, W] per core ->
    full f32 [B, L, H, W, C] (+ b_out)."""
    outT = np.concatenate([r["outT"] for r in results], axis=3)  # [2,128,B,H,L,W]
    out = np.transpose(outT, (2, 4, 3, 5, 0, 1)).reshape(B, L, H, W, C)
    return out.astype(np.float32) + np.asarray(b_out, dtype=np.float32)


def kernel(x, mask, w_qkv, w_out, b_out):
    from concourse.bass_utils import run_bass_kernel_spmd

    nc = get_nc()
    in_maps = make_in_maps(x, mask, w_qkv, w_out, b_out)
    res = run_bass_kernel_spmd(nc, in_maps, core_ids=list(range(NCORES)))
    return assemble_out(res.results, b_out)


# revision 22
# speedup vs baseline: 1.0178x; 1.0118x over previous
"""CavAttention Trainium2 kernel (fused-DVE rewrite).

Computation (per spatial location (b,h,w), L=5 "cav" slots, 8 heads x 32 dim):
  qkv = x @ w_qkv ; att = softmax_j(mask * q_i.k_j / sqrt(d)) ; o = att @ v ; out = o @ w_out + b_out

Distribution: shard the H axis (48) across the 8 cores (6 each); weights replicated.

Per-core layout: locations (b,h,w) ride the 128 SBUF partitions; (l, head, d)
rides the free axis in bf16 (DVE 2x packed mode: 0.52 ns/elem vs 1.04 at 1x).
Measured DVE cost law: 0.52 ns/elem (2x) + ~150 ns/instruction, so the
attention core is emitted as ~14 big fused ops per 128-location tile instead
of ~52 small ones: one broadcast QK mul [p,i,j,(m d)], a 5-op pairwise d-tree,
ACT exp (interleaved with the AV stage of the previous tile to hide the ACT
round-trip), a multiplicative bf16 {1,0} mask, j-reduce, fast reciprocal,
softmax normalize, one 4-dim broadcast AV mul [p,i,j,d,m], and a 3-op j-tree.
The steady state is gapless on DVE (~14.5 us/tile, PE/ACT/DMA all inside it).

The output projection runs transposed: w_out chunks are the PE stationary and
the transposed attention output streams through, producing out^T (c-major) in
PSUM. That kills the per-i bias matmuls (b_out is added on the host) and the
f32 staging copies; out^T is cast to bf16 on ACT (halving output DMA traffic)
and the host transposes back / upcasts.
"""

import numpy as np

B, L, H, W, C = 2, 5, 48, 176, 256
HEADS, DIM_HEAD = 8, 32
INNER = HEADS * DIM_HEAD  # 256
SCALE = DIM_HEAD ** -0.5
NCORES = 8
HP = H // NCORES  # 6 h-planes per core
NBH = B * HP      # 12 (b,h) blocks per core
LOCS = NBH * W    # 2112 locations per core
PTILE = 128       # locations per tile
NTILES = (LOCS + PTILE - 1) // PTILE  # 17

_cached = {}


def _pieces(s, e):
    """Split flat loc range [s,e) into (p0, b, h, w0, w1) pieces within (b,h) blocks."""
    out = []
    cur = s
    while cur < e:
        bh = cur // W
        w0 = cur % W
        w1 = min(W, w0 + (e - cur))
        out.append((cur - s, bh // HP, bh % HP, w0, w1))
        cur += w1 - w0
    return out


def _build_bass():
    import concourse.bass as bass
    import concourse.bacc as bacc
    import concourse.tile as tile
    from concourse import mybir
    from concourse.masks import make_identity

    f32 = mybir.dt.float32
    bf16 = mybir.dt.bfloat16

    nc = bacc.Bacc()
    # x arrives pre-transposed and pre-cast on the host: [cc, c, b, h, l, w]
    xT_d = nc.dram_tensor("xT", [2, 128, B, HP, L, W], bf16, kind="ExternalInput")
    # mask arrives as a bf16 multiplicative mask (1 valid / 0 masked)
    mb_d = nc.dram_tensor("mbias", [B, HP, W, L], bf16, kind="ExternalInput")
    wqkv_d = nc.dram_tensor("w_qkv", [C, 3 * INNER], bf16, kind="ExternalInput")
    wout_d = nc.dram_tensor("w_out", [INNER, C], bf16, kind="ExternalInput")
    # out^T, bf16, pre-bias: element (o, c, b, h, l, w) = out[b, l, h, w, o*128+c]
    outT_d = nc.dram_tensor("outT", [2, 128, B, HP, L, W], bf16, kind="ExternalOutput")

    with tile.TileContext(nc) as tc:
        with (
            tc.tile_pool(name="singles", bufs=1) as singles,
            tc.tile_pool(name="work", bufs=3) as work,
            tc.tile_pool(name="peri", bufs=2) as peri,
            tc.tile_pool(name="ps_t", bufs=1, space="PSUM") as ps_t,
            tc.tile_pool(name="ps_qkv", bufs=2, space="PSUM") as ps_qkv,
            tc.tile_pool(name="ps_o", bufs=1, space="PSUM") as ps_o,
        ):
            # ---- constants.  Tensors touched by PE matmuls are produced by
            #      ONE engine (ACT): PE instructions carry a single
            #      semaphore wait (walrus S3_LW limit).
            #      Order: w_qkv DMA + cast first — it gates proj(0), the head
            #      of the pipeline-fill critical path. ----
            # clock warm-up: dummy DVE + PE work while the weight DMAs are in
            # flight, so the activity monitor upshifts the core clock before
            # the first real compute (early ops otherwise run ~60% slow)
            cw = singles.tile([128, 4096], bf16)
            nc.vector.memzero(cw[:, 0:4096])
            for _ in range(3):
                nc.vector.tensor_mul(cw[:, 0:2048], cw[:, 0:2048], cw[:, 2048:4096])
            # weights arrive bf16 from the host (they are used in bf16 anyway);
            # staged through an ACT copy so PE matmul operands keep a single
            # producing engine (walrus single-wait limit)
            wqkv_l = singles.tile([128, 2, 3 * INNER], bf16)
            wqkv_sb = singles.tile([128, 2, 3 * INNER], bf16)
            for cc in range(2):
                # per-cc DMA + copy: proj(0)'s cc=0 matmuls start while the
                # cc=1 half is still in flight
                nc.sync.dma_start(out=wqkv_l[:, cc, :], in_=wqkv_d[cc * 128:(cc + 1) * 128, :])
                nc.scalar.copy(out=wqkv_sb[:, cc], in_=wqkv_l[:, cc])
            ident_l = singles.tile([128, 128], f32)
            make_identity(nc, ident_l)  # gpsimd
            wout_l = singles.tile([128, 2, C], bf16)
            nc.sync.dma_start(
                out=wout_l,
                in_=wout_d[:, :].rearrange("(cc p) n -> p cc n", cc=2),
            )

            ident = singles.tile([128, 128], bf16)
            wout_sb = singles.tile([128, 2, C], bf16)

            def setup_tail():
                """ident/wout casts — needed first by stage_b_out(0) in
                iteration 2; emitted after proj(0) so they don't sit ahead of
                the fill-critical qkv psum->sbuf copies in the ACT queue."""
                nc.scalar.copy(out=ident, in_=ident_l)
                nc.scalar.copy(out=wout_sb, in_=wout_l)

            def load(t):
                """DMA in (x already transposed+bf16 on host; partitions = C-chunk).

                The last tile holds only 64 locations; they are DUPLICATED into
                both partition halves and the upper half's Q block is l-rotated
                by 3 in proj(), so the attention stages can run with i-extent 3
                instead of 5 (partitions 0:64 cover i 0..2, 64:128 cover the
                real i 3..4; the third upper slot is a discarded duplicate)."""
                s = t * PTILE
                e = min(s + PTILE, LOCS)
                P = e - s
                pieces = _pieces(s, e)
                rot = (t == NTILES - 1)

                xt = work.tile([128, 2, L, 128], bf16, tag="xt")
                for (p0, b, h, w0, w1) in pieces:
                    for cc in range(2):
                        nc.sync.dma_start(
                            out=xt[:, cc, :, p0:p0 + (w1 - w0)],
                            in_=xT_d[cc, :, b, h, :, w0:w1],
                        )
                        if rot:
                            nc.sync.dma_start(
                                out=xt[:, cc, :, P + p0:P + p0 + (w1 - w0)],
                                in_=xT_d[cc, :, b, h, :, w0:w1],
                            )
                mbias = work.tile([128, L], bf16, tag="mbias")
                for (p0, b, h, w0, w1) in pieces:
                    nc.sync.dma_start(
                        out=mbias[p0:p0 + (w1 - w0), :],
                        in_=mb_d[b, h, w0:w1, :],
                    )
                    if rot:
                        nc.sync.dma_start(
                            out=mbias[P + p0:P + p0 + (w1 - w0), :],
                            in_=mb_d[b, h, w0:w1, :],
                        )
                return dict(P=(2 * P if rot else P), pieces=pieces, xt=xt,
                            mbias=mbias, rot=rot, P0=P)

            def proj(st):
                """qkv projection on PE + ACT psum->sbuf cast."""
                P = st["P"]
                xt = st["xt"]
                qkv_bf = work.tile([128, L, 3 * INNER], bf16, tag="qkv_bf")
                for l in range(L):
                    pq = ps_qkv.tile([128, 3 * INNER], f32, tag="psq")
                    for cc in range(2):
                        for (n0, n1) in ((0, 512), (512, 768)):
                            nc.tensor.matmul(
                                out=pq[:P, n0:n1],
                                lhsT=xt[:, cc, l, :P],
                                rhs=wqkv_sb[:, cc, n0:n1],
                                start=(cc == 0),
                                stop=(cc == 1),
                            )
                    if st["rot"]:
                        h0 = st["P0"]
                        nc.scalar.copy(out=qkv_bf[:h0, l, :], in_=pq[:h0, :])
                        # upper half: Q lands in slot (l+2)%5, K/V stay at l
                        nc.scalar.copy(out=qkv_bf[h0:P, (l + 2) % L, 0:INNER],
                                       in_=pq[h0:P, 0:INNER])
                        nc.scalar.copy(out=qkv_bf[h0:P, l, INNER:],
                                       in_=pq[h0:P, INNER:])
                    else:
                        nc.scalar.copy(out=qkv_bf[:P, l, :], in_=pq[:P, :])
                st["qkv_bf"] = qkv_bf

            def stage_a1(st):
                """QK^T on DVE; kicks off ACT exp."""
                P = st["P"]
                ni = 3 if st["rot"] else L
                st["ni"] = ni
                qkv_bf = st["qkv_bf"]
                # q[p, i, (m d)] bcast over j;  k[p, j, (m d)] bcast over i
                q_v = qkv_bf[:P, 0:ni, 0:INNER].unsqueeze(2).broadcast_to([P, ni, L, INNER])
                k_v = qkv_bf[:P, :, INNER:2 * INNER].unsqueeze(1).broadcast_to([P, ni, L, INNER])
                qk = peri.tile([128, L * L * HEADS, DIM_HEAD], bf16, tag="qk")
                qk = qk[:, 0:ni * L * HEADS, :]
                nc.vector.tensor_mul(
                    qk[:P].rearrange("p (i j m) d -> p i j (m d)", i=ni, j=L),
                    q_v, k_v,
                )
                # pairwise d-tree: 32 -> 16 -> 8 -> 4 -> 2 -> 1
                ns = ni * L * HEADS
                t16 = peri.tile([128, L * L * HEADS, 16], bf16, tag="t16")
                nc.vector.tensor_add(t16[:P, 0:ns], qk[:P, :, 0:16], qk[:P, :, 16:32])
                t8 = peri.tile([128, L * L * HEADS, 8], bf16, tag="t8")
                nc.vector.tensor_add(t8[:P, 0:ns], t16[:P, 0:ns, 0:8], t16[:P, 0:ns, 8:16])
                t4 = peri.tile([128, L * L * HEADS, 4], bf16, tag="t4")
                nc.vector.tensor_add(t4[:P, 0:ns], t8[:P, 0:ns, 0:4], t8[:P, 0:ns, 4:8])
                t2 = peri.tile([128, L * L * HEADS, 2], bf16, tag="t2")
                nc.vector.tensor_add(t2[:P, 0:ns], t4[:P, 0:ns, 0:2], t4[:P, 0:ns, 2:4])
                # final fold in f32 (exp-input precision)
                A = peri.tile([128, L, L, HEADS], f32, tag="A")
                A = A[:, 0:ni]
                nc.vector.tensor_add(
                    A[:P],
                    t2[:P, 0:ns, 0].rearrange("p (i j m) -> p i j m", i=ni, j=L),
                    t2[:P, 0:ns, 1].rearrange("p (i j m) -> p i j m", i=ni, j=L),
                )
                # exp on ACT (runs while DVE does the AV stage of tile t-2)
                eu = peri.tile([128, L, L, HEADS], bf16, tag="eu")
                eu = eu[:, 0:ni]
                nc.scalar.activation(
                    out=eu[:P], in_=A[:P], func=mybir.ActivationFunctionType.Exp,
                    scale=SCALE,
                )
                st["eu"] = eu

            def stage_a2(st):
                """softmax tail: mask, j-sum, reciprocal, normalize -> st['pw']."""
                P = st["P"]
                ni = st["ni"]
                # multiplicative {1,0} mask after exp (bf16 2x beats the f32
                # additive -1e4 logit bias, and halves the mask DMA); placed
                # here so the AV stage of tile t-2 hides the exp round-trip
                ee = work.tile([128, L, L, HEADS], bf16, tag="ee")
                ee = ee[:, 0:ni]
                nc.vector.tensor_mul(
                    ee[:P], st["eu"][:P],
                    st["mbias"][:P].unsqueeze(1).unsqueeze(3).broadcast_to([P, ni, L, HEADS]),
                )
                ssum = work.tile([128, L, HEADS], f32, tag="ssum")
                nc.vector.reduce_sum(
                    out=ssum[:P, 0:ni], in_=ee[:P].transpose([0, 1, 3, 2]),
                    axis=mybir.AxisListType.X
                )
                # ~51 ULP approx, ~5x faster than iterative reciprocal; ssum is
                # strictly positive and well inside the safe range
                sinv = work.tile([128, L, HEADS], f32, tag="sinv")
                nc.vector.reciprocal_approx_fast(out=sinv[:P, 0:ni], in_=ssum[:P, 0:ni])
                pw = work.tile([128, L, L, HEADS], bf16, tag="pw")
                pw = pw[:, 0:ni]
                nc.vector.tensor_mul(
                    pw[:P], ee[:P],
                    sinv[:P, 0:ni].unsqueeze(2).broadcast_to([P, ni, L, HEADS]),
                )
                st["pw"] = pw

            def stage_b_av(st, ia, ib):
                """attention-weighted V for i in [ia, ib): one 4-dim broadcast
                mul + 3-op j-tree.

                V rides in (d, m) order (host-permuted w_qkv columns) so the pw
                broadcast lands on a non-inner dim; w_out rows are host-permuted
                to match."""
                P = st["P"]
                ni = ib - ia
                pw = st["pw"]
                qkv_bf = st["qkv_bf"]
                # v[p, j, d, m] bcast over i;  pw[p, i, j, m] bcast over d
                v4 = (qkv_bf[:P, :, 2 * INNER:3 * INNER]
                      .rearrange("p j (d m) -> p j d m", m=HEADS)
                      .unsqueeze(1).broadcast_to([P, ni, L, DIM_HEAD, HEADS]))
                pw4 = pw[:P, ia:ib].unsqueeze(3).broadcast_to([P, ni, L, DIM_HEAD, HEADS])
                if "av" not in st:
                    st["av"] = peri.tile([128, L, L, INNER], bf16, tag="av", name="av")
                    st["s2"] = peri.tile([128, L, 2, INNER], bf16, tag="s2", name="s2")
                    st["s1"] = peri.tile([128, L, INNER], bf16, tag="s1", name="s1")
                    st["attout"] = work.tile([128, L, INNER], bf16, tag="attout", name="attout")
                av, s2, s1, attout = st["av"], st["s2"], st["s1"], st["attout"]
                nc.vector.tensor_mul(
                    av[:P, ia:ib].rearrange("p i j (d m) -> p i j d m", m=HEADS), v4, pw4,
                )
                # j-tree: 5 -> (2+2) -> 1 (+ leftover j=4)
                nc.vector.tensor_add(s2[:P, ia:ib], av[:P, ia:ib, 0:2], av[:P, ia:ib, 2:4])
                nc.vector.tensor_add(s1[:P, ia:ib], s2[:P, ia:ib, 0], s2[:P, ia:ib, 1])
                nc.vector.tensor_add(attout[:P, ia:ib], s1[:P, ia:ib], av[:P, ia:ib, 4])

            def stage_b_out(st, ia, ib):
                """PE transposes + transposed out-projection + bf16 store, for
                i in [ia, ib)."""
                P = st["P"]
                attout = st["attout"]
                # transpose attout[:, i, cc*128:(cc+1)*128] -> pt[:, i, cc, :P]
                if "pt" not in st:
                    st["pt"] = ps_t.tile([128, L, 2, 128], bf16, tag="pst", name="pt")
                    st["aoti"] = peri.tile([128, L, 2, 128], bf16, tag="aoti", name="aoti")
                    st["osb"] = peri.tile([128, 2, L, 128], bf16, tag="osb", name="osb")
                pt, aoti, osb = st["pt"], st["aoti"], st["osb"]
                for i in range(ia, ib):
                    for cc in range(2):
                        nc.tensor.transpose(
                            pt[:, i, cc, :P],
                            attout[:P, i, cc * 128:(cc + 1) * 128],
                            ident[:P, :P],
                        )
                nc.scalar.copy(out=aoti[:, ia:ib], in_=pt[:, ia:ib])
                # out^T[c_chunk, (i, loc)] = sum_cc wout[cc, c_chunk]^T @ aoti[cc]
                # accumulation groups must stay inside one 2KB PSUM bank
                # (f32 col 512 == i 4), so split i-ranges at 4.
                igroups = [(a, b) for (a, b) in ((ia, min(ib, 4)), (max(ia, 4), ib)) if a < b]
                for o in range(2):
                    po = ps_o.tile([128, L, 128], f32, tag="pso")
                    for (i0, i1) in igroups:
                        for cc in range(2):
                            nc.tensor.matmul(
                                out=po[:, i0:i1, :P],
                                lhsT=wout_sb[:, cc, o * 128:(o + 1) * 128],
                                rhs=aoti[:, i0:i1, cc, :P],
                                start=(cc == 0),
                                stop=(cc == 1),
                            )
                    nc.scalar.copy(out=osb[:, o, ia:ib], in_=po[:, ia:ib])
                if st["rot"]:
                    (p0, b, h, w0, w1) = st["pieces"][0]
                    n = w1 - w0
                    for o in range(2):
                        # lower partition half: slots ia:ib are real l ia:ib
                        nc.sync.dma_start(
                            out=outT_d[o, :, b, h, ia:ib, w0:w1],
                            in_=osb[:, o, ia:ib, 0:n],
                        )
                        # upper half: slot s (s < 2) is real l s+3; slot 2 is a
                        # discarded duplicate
                        ja, jb = ia, min(ib, 2)
                        if ja < jb:
                            nc.sync.dma_start(
                                out=outT_d[o, :, b, h, ja + 3:jb + 3, w0:w1],
                                in_=osb[:, o, ja:jb, n:2 * n],
                            )
                else:
                    for (p0, b, h, w0, w1) in st["pieces"]:
                        for o in range(2):
                            nc.sync.dma_start(
                                out=outT_d[o, :, b, h, ia:ib, w0:w1],
                                in_=osb[:, o, ia:ib, p0:p0 + (w1 - w0)],
                            )

            # 3-deep software pipeline: per iteration t issue
            #   load(t)+proj(t)    DMA + PE qkv + ACT cast
            #   stage_a1(t-1)      DVE qk -> ACT exp
            #   stage_b(t-2)       DVE av (covers the exp round-trip)
            #   stage_a2(t-1)      DVE softmax tail
            #   stage_b_out(t-2)   PE transpose + out-proj; ACT cast; DMA store
            sts = {}
            for t in range(NTILES + 2):
                if t < NTILES:
                    sts[t] = load(t)
                    proj(sts[t])
                    if t == 0:
                        setup_tail()
                if 0 <= t - 1 < NTILES:
                    stage_a1(sts[t - 1])
                if 0 <= t - 2 < NTILES:
                    if t - 2 == NTILES - 1:
                        # last tile: i-extent 3 (rotated duplicate layout) and
                        # per-i backend chunks so its PE/ACT/DMA tail overlaps
                        # the tail of the DVE work (pipeline drain)
                        for i in range(3):
                            stage_b_av(sts[t - 2], i, i + 1)
                            stage_b_out(sts[t - 2], i, i + 1)
                    else:
                        stage_b_av(sts[t - 2], 0, 5)
                if 0 <= t - 1 < NTILES:
                    stage_a2(sts[t - 1])
                if 0 <= t - 2 < NTILES:
                    if t - 2 != NTILES - 1:
                        stage_b_out(sts[t - 2], 0, 5)
                    del sts[t - 2]
    nc.finalize()  # Bacc.compile(): legalize multi-wait instructions, alloc regs
    return nc


def get_nc():
    if "nc" not in _cached:
        _cached["nc"] = _build_bass()
    return _cached["nc"]


def make_in_maps(x, mask, w_qkv, w_out, b_out):
    """Host-side shard + repack: x is transposed to [cc, c, b, h, l, w] and
    cast to bf16; the mask becomes an f32 additive logit bias."""
    import ml_dtypes

    x = np.asarray(x, dtype=np.float32)
    mask = np.asarray(mask)
    w_qkv = np.ascontiguousarray(np.asarray(w_qkv), dtype=np.float32)
    w_out = np.ascontiguousarray(np.asarray(w_out), dtype=np.float32)

    # permute V's output columns (m,d)->(d,m) and w_out's rows to match, so
    # the device-side pw broadcast is never on the innermost dim
    wv = w_qkv[:, 2 * INNER:].reshape(C, HEADS, DIM_HEAD).transpose(0, 2, 1).reshape(C, INNER)
    w_qkv = np.ascontiguousarray(
        np.concatenate([w_qkv[:, :2 * INNER], wv], axis=1).astype(ml_dtypes.bfloat16)
    )
    w_out = np.ascontiguousarray(
        w_out.reshape(HEADS, DIM_HEAD, C).transpose(1, 0, 2).reshape(INNER, C)
        .astype(ml_dtypes.bfloat16)
    )

    # [B, L, H, W, C] -> [C, B, H, L, W] -> [2, 128, B, H, L, W] bf16
    xT = np.transpose(x, (4, 0, 2, 1, 3)).astype(ml_dtypes.bfloat16)
    xT = np.ascontiguousarray(xT.reshape(2, 128, B, H, L, W))
    # [B, H, W, 1, L] -> bf16 multiplicative mask [B, H, W, L]
    mb = np.ascontiguousarray(
        (mask[:, :, :, 0, :] != 0).astype(ml_dtypes.bfloat16)
    )

    in_maps = []
    for k in range(NCORES):
        h0, h1 = k * HP, (k + 1) * HP
        in_maps.append({
            "xT": np.ascontiguousarray(xT[:, :, :, h0:h1]),
            "mbias": np.ascontiguousarray(mb[:, h0:h1]),
            "w_qkv": w_qkv,
            "w_out": w_out,
        })
    return in_maps


def assemble_out(results, b_out):
    """Host-side unshard: out^T bf16 [2, 128, B, HP, L, W] per core ->
    full f32 [B, L, H, W, C] (+ b_out)."""
    outT = np.concatenate([r["outT"] for r in results], axis=3)  # [2,128,B,H,L,W]
    out = np.transpose(outT, (2, 4, 3, 5, 0, 1)).reshape(B, L, H, W, C)
    return out.astype(np.float32) + np.asarray(b_out, dtype=np.float32)


def kernel(x, mask, w_qkv, w_out, b_out):
    from concourse.bass_utils import run_bass_kernel_spmd

    nc = get_nc()
    in_maps = make_in_maps(x, mask, w_qkv, w_out, b_out)
    res = run_bass_kernel_spmd(nc, in_maps, core_ids=list(range(NCORES)))
    return assemble_out(res.results, b_out)


# revision 23
# speedup vs baseline: 1.0185x; 1.0007x over previous
"""CavAttention Trainium2 kernel (fused-DVE rewrite).

Computation (per spatial location (b,h,w), L=5 "cav" slots, 8 heads x 32 dim):
  qkv = x @ w_qkv ; att = softmax_j(mask * q_i.k_j / sqrt(d)) ; o = att @ v ; out = o @ w_out + b_out

Distribution: shard the H axis (48) across the 8 cores (6 each); weights replicated.

Per-core layout: locations (b,h,w) ride the 128 SBUF partitions; (l, head, d)
rides the free axis in bf16 (DVE 2x packed mode: 0.52 ns/elem vs 1.04 at 1x).
Measured DVE cost law: 0.52 ns/elem (2x) + ~150 ns/instruction, so the
attention core is emitted as ~14 big fused ops per 128-location tile instead
of ~52 small ones: one broadcast QK mul [p,i,j,(m d)], a 5-op pairwise d-tree,
ACT exp (interleaved with the AV stage of the previous tile to hide the ACT
round-trip), a multiplicative bf16 {1,0} mask, j-reduce, fast reciprocal,
softmax normalize, one 4-dim broadcast AV mul [p,i,j,d,m], and a 3-op j-tree.
The steady state is gapless on DVE (~14.5 us/tile, PE/ACT/DMA all inside it).

The output projection runs transposed: w_out chunks are the PE stationary and
the transposed attention output streams through, producing out^T (c-major) in
PSUM. That kills the per-i bias matmuls (b_out is added on the host) and the
f32 staging copies; out^T is cast to bf16 on ACT (halving output DMA traffic)
and the host transposes back / upcasts.
"""

import numpy as np

B, L, H, W, C = 2, 5, 48, 176, 256
HEADS, DIM_HEAD = 8, 32
INNER = HEADS * DIM_HEAD  # 256
SCALE = DIM_HEAD ** -0.5
NCORES = 8
HP = H // NCORES  # 6 h-planes per core
NBH = B * HP      # 12 (b,h) blocks per core
LOCS = NBH * W    # 2112 locations per core
PTILE = 128       # locations per tile
NTILES = (LOCS + PTILE - 1) // PTILE  # 17

_cached = {}


def _pieces(s, e):
    """Split flat loc range [s,e) into (p0, b, h, w0, w1) pieces within (b,h) blocks."""
    out = []
    cur = s
    while cur < e:
        bh = cur // W
        w0 = cur % W
        w1 = min(W, w0 + (e - cur))
        out.append((cur - s, bh // HP, bh % HP, w0, w1))
        cur += w1 - w0
    return out


def _build_bass():
    import concourse.bass as bass
    import concourse.bacc as bacc
    import concourse.tile as tile
    from concourse import mybir
    from concourse.masks import make_identity

    f32 = mybir.dt.float32
    bf16 = mybir.dt.bfloat16

    nc = bacc.Bacc()
    # x arrives pre-transposed and pre-cast on the host: [cc, c, b, h, l, w]
    xT_d = nc.dram_tensor("xT", [2, 128, B, HP, L, W], bf16, kind="ExternalInput")
    # mask arrives as a bf16 multiplicative mask (1 valid / 0 masked)
    mb_d = nc.dram_tensor("mbias", [B, HP, W, L], bf16, kind="ExternalInput")
    wqkv_d = nc.dram_tensor("w_qkv", [C, 3 * INNER], bf16, kind="ExternalInput")
    wout_d = nc.dram_tensor("w_out", [INNER, C], bf16, kind="ExternalInput")
    # out^T, bf16, pre-bias: element (o, c, b, h, l, w) = out[b, l, h, w, o*128+c]
    outT_d = nc.dram_tensor("outT", [2, 128, B, HP, L, W], bf16, kind="ExternalOutput")

    with tile.TileContext(nc) as tc:
        with (
            tc.tile_pool(name="singles", bufs=1) as singles,
            tc.tile_pool(name="work", bufs=3) as work,
            tc.tile_pool(name="peri", bufs=2) as peri,
            tc.tile_pool(name="ps_t", bufs=1, space="PSUM") as ps_t,
            tc.tile_pool(name="ps_qkv", bufs=2, space="PSUM") as ps_qkv,
            tc.tile_pool(name="ps_o", bufs=1, space="PSUM") as ps_o,
        ):
            # ---- constants.  Tensors touched by PE matmuls are produced by
            #      ONE engine (ACT): PE instructions carry a single
            #      semaphore wait (walrus S3_LW limit).
            #      Order: w_qkv DMA + cast first — it gates proj(0), the head
            #      of the pipeline-fill critical path. ----
            # clock warm-up: dummy DVE + PE work while the weight DMAs are in
            # flight, so the activity monitor upshifts the core clock before
            # the first real compute (early ops otherwise run ~60% slow)
            cw = singles.tile([128, 4096], bf16)
            nc.vector.memzero(cw[:, 0:4096])
            for _ in range(3):
                nc.vector.tensor_mul(cw[:, 0:2048], cw[:, 0:2048], cw[:, 2048:4096])
            # weights arrive bf16 from the host (they are used in bf16 anyway);
            # staged through an ACT copy so PE matmul operands keep a single
            # producing engine (walrus single-wait limit)
            wqkv_l = singles.tile([128, 2, 3 * INNER], bf16)
            wqkv_sb = singles.tile([128, 2, 3 * INNER], bf16)
            for cc in range(2):
                # per-cc DMA + copy: proj(0)'s cc=0 matmuls start while the
                # cc=1 half is still in flight
                nc.sync.dma_start(out=wqkv_l[:, cc, :], in_=wqkv_d[cc * 128:(cc + 1) * 128, :])
                nc.scalar.copy(out=wqkv_sb[:, cc], in_=wqkv_l[:, cc])
            ident_l = singles.tile([128, 128], f32)
            make_identity(nc, ident_l)  # gpsimd
            wout_l = singles.tile([128, 2, C], bf16)
            nc.sync.dma_start(
                out=wout_l,
                in_=wout_d[:, :].rearrange("(cc p) n -> p cc n", cc=2),
            )

            ident = singles.tile([128, 128], bf16)
            wout_sb = singles.tile([128, 2, C], bf16)

            def setup_tail():
                """ident/wout casts — needed first by stage_b_out(0) in
                iteration 2; emitted after proj(0) so they don't sit ahead of
                the fill-critical qkv psum->sbuf copies in the ACT queue."""
                nc.scalar.copy(out=ident, in_=ident_l)
                nc.scalar.copy(out=wout_sb, in_=wout_l)

            def load(t):
                """DMA in (x already transposed+bf16 on host; partitions = C-chunk).

                The last tile holds only 64 locations; they are DUPLICATED into
                both partition halves and the upper half's Q block is l-rotated
                by 3 in proj(), so the attention stages can run with i-extent 3
                instead of 5 (partitions 0:64 cover i 0..2, 64:128 cover the
                real i 3..4; the third upper slot is a discarded duplicate)."""
                s = t * PTILE
                e = min(s + PTILE, LOCS)
                P = e - s
                pieces = _pieces(s, e)
                rot = (t == NTILES - 1)

                xt = work.tile([128, 2, L, 128], bf16, tag="xt")
                for (p0, b, h, w0, w1) in pieces:
                    for cc in range(2):
                        nc.sync.dma_start(
                            out=xt[:, cc, :, p0:p0 + (w1 - w0)],
                            in_=xT_d[cc, :, b, h, :, w0:w1],
                        )
                        if rot:
                            nc.sync.dma_start(
                                out=xt[:, cc, :, P + p0:P + p0 + (w1 - w0)],
                                in_=xT_d[cc, :, b, h, :, w0:w1],
                            )
                mbias = work.tile([128, L], bf16, tag="mbias")
                for (p0, b, h, w0, w1) in pieces:
                    nc.sync.dma_start(
                        out=mbias[p0:p0 + (w1 - w0), :],
                        in_=mb_d[b, h, w0:w1, :],
                    )
                    if rot:
                        nc.sync.dma_start(
                            out=mbias[P + p0:P + p0 + (w1 - w0), :],
                            in_=mb_d[b, h, w0:w1, :],
                        )
                return dict(P=(2 * P if rot else P), pieces=pieces, xt=xt,
                            mbias=mbias, rot=rot, P0=P)

            def proj(st):
                """qkv projection on PE + ACT psum->sbuf cast."""
                P = st["P"]
                xt = st["xt"]
                qkv_bf = work.tile([128, L, 3 * INNER], bf16, tag="qkv_bf")
                for l in range(L):
                    pq = ps_qkv.tile([128, 3 * INNER], f32, tag="psq")
                    for cc in range(2):
                        for (n0, n1) in ((0, 512), (512, 768)):
                            nc.tensor.matmul(
                                out=pq[:P, n0:n1],
                                lhsT=xt[:, cc, l, :P],
                                rhs=wqkv_sb[:, cc, n0:n1],
                                start=(cc == 0),
                                stop=(cc == 1),
                            )
                    if st["rot"]:
                        h0 = st["P0"]
                        nc.scalar.copy(out=qkv_bf[:h0, l, :], in_=pq[:h0, :])
                        # upper half: Q lands in slot (l+2)%5, K/V stay at l
                        nc.scalar.copy(out=qkv_bf[h0:P, (l + 2) % L, 0:INNER],
                                       in_=pq[h0:P, 0:INNER])
                        nc.scalar.copy(out=qkv_bf[h0:P, l, INNER:],
                                       in_=pq[h0:P, INNER:])
                    else:
                        nc.scalar.copy(out=qkv_bf[:P, l, :], in_=pq[:P, :])
                st["qkv_bf"] = qkv_bf

            def stage_a1(st):
                """QK^T on DVE; kicks off ACT exp."""
                P = st["P"]
                ni = 3 if st["rot"] else L
                st["ni"] = ni
                qkv_bf = st["qkv_bf"]
                # q[p, i, (m d)] bcast over j;  k[p, j, (m d)] bcast over i
                q_v = qkv_bf[:P, 0:ni, 0:INNER].unsqueeze(2).broadcast_to([P, ni, L, INNER])
                k_v = qkv_bf[:P, :, INNER:2 * INNER].unsqueeze(1).broadcast_to([P, ni, L, INNER])
                qk = peri.tile([128, L * L * HEADS, DIM_HEAD], bf16, tag="qk")
                qk = qk[:, 0:ni * L * HEADS, :]
                nc.vector.tensor_mul(
                    qk[:P].rearrange("p (i j m) d -> p i j (m d)", i=ni, j=L),
                    q_v, k_v,
                )
                # pairwise d-tree: 32 -> 16 -> 8 -> 4 -> 2 -> 1
                ns = ni * L * HEADS
                t16 = peri.tile([128, L * L * HEADS, 16], bf16, tag="t16")
                nc.vector.tensor_add(t16[:P, 0:ns], qk[:P, :, 0:16], qk[:P, :, 16:32])
                t8 = peri.tile([128, L * L * HEADS, 8], bf16, tag="t8")
                nc.vector.tensor_add(t8[:P, 0:ns], t16[:P, 0:ns, 0:8], t16[:P, 0:ns, 8:16])
                t4 = peri.tile([128, L * L * HEADS, 4], bf16, tag="t4")
                nc.vector.tensor_add(t4[:P, 0:ns], t8[:P, 0:ns, 0:4], t8[:P, 0:ns, 4:8])
                t2 = peri.tile([128, L * L * HEADS, 2], bf16, tag="t2")
                nc.vector.tensor_add(t2[:P, 0:ns], t4[:P, 0:ns, 0:2], t4[:P, 0:ns, 2:4])
                # final fold in f32 (exp-input precision)
                A = peri.tile([128, L, L, HEADS], f32, tag="A")
                A = A[:, 0:ni]
                nc.vector.tensor_add(
                    A[:P],
                    t2[:P, 0:ns, 0].rearrange("p (i j m) -> p i j m", i=ni, j=L),
                    t2[:P, 0:ns, 1].rearrange("p (i j m) -> p i j m", i=ni, j=L),
                )
                # exp on ACT (runs while DVE does the AV stage of tile t-2)
                eu = peri.tile([128, L, L, HEADS], bf16, tag="eu")
                eu = eu[:, 0:ni]
                nc.scalar.activation(
                    out=eu[:P], in_=A[:P], func=mybir.ActivationFunctionType.Exp,
                    scale=SCALE,
                )
                st["eu"] = eu

            def stage_a2(st):
                """softmax tail: mask, j-sum, reciprocal, normalize -> st['pw']."""
                P = st["P"]
                ni = st["ni"]
                # multiplicative {1,0} mask after exp (bf16 2x beats the f32
                # additive -1e4 logit bias, and halves the mask DMA); placed
                # here so the AV stage of tile t-2 hides the exp round-trip
                ee = work.tile([128, L, L, HEADS], bf16, tag="ee")
                ee = ee[:, 0:ni]
                nc.vector.tensor_mul(
                    ee[:P], st["eu"][:P],
                    st["mbias"][:P].unsqueeze(1).unsqueeze(3).broadcast_to([P, ni, L, HEADS]),
                )
                ssum = work.tile([128, L, HEADS], f32, tag="ssum")
                nc.vector.reduce_sum(
                    out=ssum[:P, 0:ni], in_=ee[:P].transpose([0, 1, 3, 2]),
                    axis=mybir.AxisListType.X
                )
                # ~51 ULP approx, ~5x faster than iterative reciprocal; ssum is
                # strictly positive and well inside the safe range
                sinv = work.tile([128, L, HEADS], f32, tag="sinv")
                nc.vector.reciprocal_approx_fast(out=sinv[:P, 0:ni], in_=ssum[:P, 0:ni])
                pw = work.tile([128, L, L, HEADS], bf16, tag="pw")
                pw = pw[:, 0:ni]
                nc.vector.tensor_mul(
                    pw[:P], ee[:P],
                    sinv[:P, 0:ni].unsqueeze(2).broadcast_to([P, ni, L, HEADS]),
                )
                st["pw"] = pw

            def stage_b_av(st, ia, ib):
                """attention-weighted V for i in [ia, ib): one 4-dim broadcast
                mul + 3-op j-tree.

                V rides in (d, m) order (host-permuted w_qkv columns) so the pw
                broadcast lands on a non-inner dim; w_out rows are host-permuted
                to match."""
                P = st["P"]
                ni = ib - ia
                pw = st["pw"]
                qkv_bf = st["qkv_bf"]
                # v[p, j, d, m] bcast over i;  pw[p, i, j, m] bcast over d
                v4 = (qkv_bf[:P, :, 2 * INNER:3 * INNER]
                      .rearrange("p j (d m) -> p j d m", m=HEADS)
                      .unsqueeze(1).broadcast_to([P, ni, L, DIM_HEAD, HEADS]))
                pw4 = pw[:P, ia:ib].unsqueeze(3).broadcast_to([P, ni, L, DIM_HEAD, HEADS])
                if "av" not in st:
                    st["av"] = peri.tile([128, L, L, INNER], bf16, tag="av", name="av")
                    st["s2"] = peri.tile([128, L, 2, INNER], bf16, tag="s2", name="s2")
                    st["s1"] = peri.tile([128, L, INNER], bf16, tag="s1", name="s1")
                    st["attout"] = work.tile([128, L, INNER], bf16, tag="attout", name="attout")
                av, s2, s1, attout = st["av"], st["s2"], st["s1"], st["attout"]
                nc.vector.tensor_mul(
                    av[:P, ia:ib].rearrange("p i j (d m) -> p i j d m", m=HEADS), v4, pw4,
                )
                # j-tree: 5 -> (2+2) -> 1 (+ leftover j=4)
                nc.vector.tensor_add(s2[:P, ia:ib], av[:P, ia:ib, 0:2], av[:P, ia:ib, 2:4])
                nc.vector.tensor_add(s1[:P, ia:ib], s2[:P, ia:ib, 0], s2[:P, ia:ib, 1])
                nc.vector.tensor_add(attout[:P, ia:ib], s1[:P, ia:ib], av[:P, ia:ib, 4])

            def stage_b_out(st, ia, ib, do_dma=True):
                """PE transposes + transposed out-projection + bf16 store, for
                i in [ia, ib)."""
                P = st["P"]
                attout = st["attout"]
                # transpose attout[:, i, cc*128:(cc+1)*128] -> pt[:, i, cc, :P]
                if "pt" not in st:
                    st["pt"] = ps_t.tile([128, L, 2, 128], bf16, tag="pst", name="pt")
                    st["aoti"] = peri.tile([128, L, 2, 128], bf16, tag="aoti", name="aoti")
                    st["osb"] = peri.tile([128, 2, L, 128], bf16, tag="osb", name="osb")
                pt, aoti, osb = st["pt"], st["aoti"], st["osb"]
                for i in range(ia, ib):
                    for cc in range(2):
                        nc.tensor.transpose(
                            pt[:, i, cc, :P],
                            attout[:P, i, cc * 128:(cc + 1) * 128],
                            ident[:P, :P],
                        )
                nc.scalar.copy(out=aoti[:, ia:ib], in_=pt[:, ia:ib])
                # out^T[c_chunk, (i, loc)] = sum_cc wout[cc, c_chunk]^T @ aoti[cc]
                # accumulation groups must stay inside one 2KB PSUM bank
                # (f32 col 512 == i 4), so split i-ranges at 4.
                igroups = [(a, b) for (a, b) in ((ia, min(ib, 4)), (max(ia, 4), ib)) if a < b]
                for o in range(2):
                    po = ps_o.tile([128, L, 128], f32, tag="pso")
                    for (i0, i1) in igroups:
                        for cc in range(2):
                            nc.tensor.matmul(
                                out=po[:, i0:i1, :P],
                                lhsT=wout_sb[:, cc, o * 128:(o + 1) * 128],
                                rhs=aoti[:, i0:i1, cc, :P],
                                start=(cc == 0),
                                stop=(cc == 1),
                            )
                    nc.scalar.copy(out=osb[:, o, ia:ib], in_=po[:, ia:ib])
                if not do_dma:
                    return
                if st["rot"]:
                    (p0, b, h, w0, w1) = st["pieces"][0]
                    n = w1 - w0
                    for o in range(2):
                        # lower partition half: slots ia:ib are real l ia:ib
                        nc.sync.dma_start(
                            out=outT_d[o, :, b, h, ia:ib, w0:w1],
                            in_=osb[:, o, ia:ib, 0:n],
                        )
                        # upper half: slot s (s < 2) is real l s+3; slot 2 is a
                        # discarded duplicate
                        ja, jb = ia, min(ib, 2)
                        if ja < jb:
                            nc.sync.dma_start(
                                out=outT_d[o, :, b, h, ja + 3:jb + 3, w0:w1],
                                in_=osb[:, o, ja:jb, n:2 * n],
                            )
                else:
                    for (p0, b, h, w0, w1) in st["pieces"]:
                        for o in range(2):
                            nc.sync.dma_start(
                                out=outT_d[o, :, b, h, ia:ib, w0:w1],
                                in_=osb[:, o, ia:ib, p0:p0 + (w1 - w0)],
                            )

            # 3-deep software pipeline: per iteration t issue
            #   load(t)+proj(t)    DMA + PE qkv + ACT cast
            #   stage_a1(t-1)      DVE qk -> ACT exp
            #   stage_b(t-2)       DVE av (covers the exp round-trip)
            #   stage_a2(t-1)      DVE softmax tail
            #   stage_b_out(t-2)   PE transpose + out-proj; ACT cast; DMA store
            sts = {}
            for t in range(NTILES + 2):
                if t < NTILES:
                    sts[t] = load(t)
                    proj(sts[t])
                    if t == 0:
                        setup_tail()
                if 0 <= t - 1 < NTILES:
                    stage_a1(sts[t - 1])
                if 0 <= t - 2 < NTILES:
                    if t - 2 == NTILES - 1:
                        # last tile: i-extent 3 (rotated duplicate layout) and
                        # per-i backend chunks so its PE/ACT/DMA tail overlaps
                        # the tail of the DVE work; the output DMAs are
                        # coarsened into one final batch (10 small late DMAs
                        # serialized ~3us of drain otherwise)
                        for i in range(3):
                            stage_b_av(sts[t - 2], i, i + 1)
                            stage_b_out(sts[t - 2], i, i + 1, do_dma=False)
                        stage_b_out(sts[t - 2], 0, 3, do_dma=True)
                    else:
                        stage_b_av(sts[t - 2], 0, 5)
                if 0 <= t - 1 < NTILES:
                    stage_a2(sts[t - 1])
                if 0 <= t - 2 < NTILES:
                    if t - 2 != NTILES - 1:
                        stage_b_out(sts[t - 2], 0, 5)
                    del sts[t - 2]
    nc.finalize()  # Bacc.compile(): legalize multi-wait instructions, alloc regs
    return nc


def get_nc():
    if "nc" not in _cached:
        _cached["nc"] = _build_bass()
    return _cached["nc"]


def make_in_maps(x, mask, w_qkv, w_out, b_out):
    """Host-side shard + repack: x is transposed to [cc, c, b, h, l, w] and
    cast to bf16; the mask becomes an f32 additive logit bias."""
    import ml_dtypes

    x = np.asarray(x, dtype=np.float32)
    mask = np.asarray(mask)
    w_qkv = np.ascontiguousarray(np.asarray(w_qkv), dtype=np.float32)
    w_out = np.ascontiguousarray(np.asarray(w_out), dtype=np.float32)

    # permute V's output columns (m,d)->(d,m) and w_out's rows to match, so
    # the device-side pw broadcast is never on the innermost dim
    wv = w_qkv[:, 2 * INNER:].reshape(C, HEADS, DIM_HEAD).transpose(0, 2, 1).reshape(C, INNER)
    w_qkv = np.ascontiguousarray(
        np.concatenate([w_qkv[:, :2 * INNER], wv], axis=1).astype(ml_dtypes.bfloat16)
    )
    w_out = np.ascontiguousarray(
        w_out.reshape(HEADS, DIM_HEAD, C).transpose(1, 0, 2).reshape(INNER, C)
        .astype(ml_dtypes.bfloat16)
    )

    # [B, L, H, W, C] -> [C, B, H, L, W] -> [2, 128, B, H, L, W] bf16
    xT = np.transpose(x, (4, 0, 2, 1, 3)).astype(ml_dtypes.bfloat16)
    xT = np.ascontiguousarray(xT.reshape(2, 128, B, H, L, W))
    # [B, H, W, 1, L] -> bf16 multiplicative mask [B, H, W, L]
    mb = np.ascontiguousarray(
        (mask[:, :, :, 0, :] != 0).astype(ml_dtypes.bfloat16)
    )

    in_maps = []
    for k in range(NCORES):
        h0, h1 = k * HP, (k + 1) * HP
        in_maps.append({
            "xT": np.ascontiguousarray(xT[:, :, :, h0:h1]),
            "mbias": np.ascontiguousarray(mb[:, h0:h1]),
            "w_qkv": w_qkv,
            "w_out": w_out,
        })
    return in_maps


def assemble_out(results, b_out):
    """Host-side unshard: out^T bf16 [2, 128, B, HP, L, W] per core ->
    full f32 [B, L, H, W, C] (+ b_out)."""
    outT = np.concatenate([r["outT"] for r in results], axis=3)  # [2,128,B,H,L,W]
    out = np.transpose(outT, (2, 4, 3, 5, 0, 1)).reshape(B, L, H, W, C)
    return out.astype(np.float32) + np.asarray(b_out, dtype=np.float32)


def kernel(x, mask, w_qkv, w_out, b_out):
    from concourse.bass_utils import run_bass_kernel_spmd

    nc = get_nc()
    in_maps = make_in_maps(x, mask, w_qkv, w_out, b_out)
    res = run_bass_kernel_spmd(nc, in_maps, core_ids=list(range(NCORES)))
    return assemble_out(res.results, b_out)


# revision 25
# speedup vs baseline: 1.0294x; 1.0108x over previous
"""CavAttention Trainium2 kernel (fused-DVE rewrite).

Computation (per spatial location (b,h,w), L=5 "cav" slots, 8 heads x 32 dim):
  qkv = x @ w_qkv ; att = softmax_j(mask * q_i.k_j / sqrt(d)) ; o = att @ v ; out = o @ w_out + b_out

Distribution: shard the H axis (48) across the 8 cores (6 each); weights replicated.

Per-core layout: locations (b,h,w) ride the 128 SBUF partitions; (l, head, d)
rides the free axis in bf16 (DVE 2x packed mode: 0.52 ns/elem vs 1.04 at 1x).
Measured DVE cost law: 0.52 ns/elem (2x) + ~150 ns/instruction, so the
attention core is emitted as ~14 big fused ops per 128-location tile instead
of ~52 small ones: one broadcast QK mul [p,i,j,(m d)], a 5-op pairwise d-tree,
ACT exp (interleaved with the AV stage of the previous tile to hide the ACT
round-trip), a multiplicative bf16 {1,0} mask, j-reduce, fast reciprocal,
softmax normalize, one 4-dim broadcast AV mul [p,i,j,d,m], and a 3-op j-tree.
The steady state is gapless on DVE (~14.5 us/tile, PE/ACT/DMA all inside it).

The output projection runs transposed: w_out chunks are the PE stationary and
the transposed attention output streams through, producing out^T (c-major) in
PSUM. That kills the per-i bias matmuls (b_out is added on the host) and the
f32 staging copies; out^T is cast to bf16 on ACT (halving output DMA traffic)
and the host transposes back / upcasts.
"""

import numpy as np

B, L, H, W, C = 2, 5, 48, 176, 256
HEADS, DIM_HEAD = 8, 32
INNER = HEADS * DIM_HEAD  # 256
SCALE = DIM_HEAD ** -0.5
NCORES = 8
HP = H // NCORES  # 6 h-planes per core
NBH = B * HP      # 12 (b,h) blocks per core
LOCS = NBH * W    # 2112 locations per core
PTILE = 128       # locations per tile
NTILES = (LOCS + PTILE - 1) // PTILE  # 17

_cached = {}


def _pieces(s, e):
    """Split flat loc range [s,e) into (p0, b, h, w0, w1) pieces within (b,h) blocks."""
    out = []
    cur = s
    while cur < e:
        bh = cur // W
        w0 = cur % W
        w1 = min(W, w0 + (e - cur))
        out.append((cur - s, bh // HP, bh % HP, w0, w1))
        cur += w1 - w0
    return out


def _build_bass():
    import concourse.bass as bass
    import concourse.bacc as bacc
    import concourse.tile as tile
    from concourse import mybir
    from concourse.masks import make_identity

    f32 = mybir.dt.float32
    bf16 = mybir.dt.bfloat16

    nc = bacc.Bacc()
    # x arrives pre-transposed and pre-cast on the host: [cc, c, b, h, l, w]
    xT_d = nc.dram_tensor("xT", [2, 128, B, HP, L, W], bf16, kind="ExternalInput")
    # mask arrives as a bf16 multiplicative mask (1 valid / 0 masked)
    mb_d = nc.dram_tensor("mbias", [B, HP, W, L], bf16, kind="ExternalInput")
    wqkv_d = nc.dram_tensor("w_qkv", [C, 3 * INNER], bf16, kind="ExternalInput")
    wout_d = nc.dram_tensor("w_out", [INNER, C], bf16, kind="ExternalInput")
    # out^T, bf16, pre-bias: element (o, c, b, h, l, w) = out[b, l, h, w, o*128+c]
    outT_d = nc.dram_tensor("outT", [2, 128, B, HP, L, W], bf16, kind="ExternalOutput")

    with tile.TileContext(nc) as tc:
        with (
            tc.tile_pool(name="singles", bufs=1) as singles,
            tc.tile_pool(name="work", bufs=3) as work,
            tc.tile_pool(name="peri", bufs=2) as peri,
            tc.tile_pool(name="ps_t", bufs=1, space="PSUM") as ps_t,
            tc.tile_pool(name="ps_qkv", bufs=2, space="PSUM") as ps_qkv,
            tc.tile_pool(name="ps_o", bufs=1, space="PSUM") as ps_o,
        ):
            # ---- constants.  Tensors touched by PE matmuls are produced by
            #      ONE engine (ACT): PE instructions carry a single
            #      semaphore wait (walrus S3_LW limit).
            #      Order: w_qkv DMA + cast first — it gates proj(0), the head
            #      of the pipeline-fill critical path. ----
            # clock warm-up: dummy DVE + PE work while the weight DMAs are in
            # flight, so the activity monitor upshifts the core clock before
            # the first real compute (early ops otherwise run ~60% slow)
            cw = singles.tile([128, 4096], bf16)
            nc.vector.memzero(cw[:, 0:4096])
            for _ in range(3):
                nc.vector.tensor_mul(cw[:, 0:2048], cw[:, 0:2048], cw[:, 2048:4096])
            # weights arrive bf16 from the host (they are used in bf16 anyway);
            # staged through an ACT copy so PE matmul operands keep a single
            # producing engine (walrus single-wait limit)
            wqkv_l = singles.tile([128, 2, 3 * INNER], bf16)
            wqkv_sb = singles.tile([128, 2, 3 * INNER], bf16)
            for cc in range(2):
                # per-cc DMA + copy: proj(0)'s cc=0 matmuls start while the
                # cc=1 half is still in flight
                nc.sync.dma_start(out=wqkv_l[:, cc, :], in_=wqkv_d[cc * 128:(cc + 1) * 128, :])
                nc.scalar.copy(out=wqkv_sb[:, cc], in_=wqkv_l[:, cc])
            ident_l = singles.tile([128, 128], f32)
            make_identity(nc, ident_l)  # gpsimd
            wout_l = singles.tile([128, 2, C], bf16)
            nc.sync.dma_start(
                out=wout_l,
                in_=wout_d[:, :].rearrange("(cc p) n -> p cc n", cc=2),
            )

            ident = singles.tile([128, 128], bf16)
            wout_sb = singles.tile([128, 2, C], bf16)

            def setup_tail():
                """ident/wout casts — needed first by stage_b_out(0) in
                iteration 2; emitted after proj(0) so they don't sit ahead of
                the fill-critical qkv psum->sbuf copies in the ACT queue."""
                nc.scalar.copy(out=ident, in_=ident_l)
                nc.scalar.copy(out=wout_sb, in_=wout_l)

            def load(t):
                """DMA in (x already transposed+bf16 on host; partitions = C-chunk).

                The last tile holds only 64 locations; they are DUPLICATED into
                both partition halves and the upper half's Q block is l-rotated
                by 3 in proj(), so the attention stages can run with i-extent 3
                instead of 5 (partitions 0:64 cover i 0..2, 64:128 cover the
                real i 3..4; the third upper slot is a discarded duplicate)."""
                s = t * PTILE
                e = min(s + PTILE, LOCS)
                P = e - s
                pieces = _pieces(s, e)
                rot = (t == NTILES - 1)

                xt = work.tile([128, 2, L, 128], bf16, tag="xt")
                for (p0, b, h, w0, w1) in pieces:
                    for cc in range(2):
                        nc.sync.dma_start(
                            out=xt[:, cc, :, p0:p0 + (w1 - w0)],
                            in_=xT_d[cc, :, b, h, :, w0:w1],
                        )
                        if rot:
                            nc.sync.dma_start(
                                out=xt[:, cc, :, P + p0:P + p0 + (w1 - w0)],
                                in_=xT_d[cc, :, b, h, :, w0:w1],
                            )
                mbias = work.tile([128, L], bf16, tag="mbias")
                for (p0, b, h, w0, w1) in pieces:
                    nc.sync.dma_start(
                        out=mbias[p0:p0 + (w1 - w0), :],
                        in_=mb_d[b, h, w0:w1, :],
                    )
                    if rot:
                        nc.sync.dma_start(
                            out=mbias[P + p0:P + p0 + (w1 - w0), :],
                            in_=mb_d[b, h, w0:w1, :],
                        )
                return dict(P=(2 * P if rot else P), pieces=pieces, xt=xt,
                            mbias=mbias, rot=rot, P0=P)

            def proj(st):
                """qkv projection on PE + ACT psum->sbuf cast."""
                P = st["P"]
                xt = st["xt"]
                qkv_bf = work.tile([128, L, 3 * INNER], bf16, tag="qkv_bf")
                for l in range(L):
                    pq = ps_qkv.tile([128, 3 * INNER], f32, tag="psq")
                    for cc in range(2):
                        for (n0, n1) in ((0, 512), (512, 768)):
                            nc.tensor.matmul(
                                out=pq[:P, n0:n1],
                                lhsT=xt[:, cc, l, :P],
                                rhs=wqkv_sb[:, cc, n0:n1],
                                start=(cc == 0),
                                stop=(cc == 1),
                            )
                    if st["rot"]:
                        h0 = st["P0"]
                        nc.scalar.copy(out=qkv_bf[:h0, l, :], in_=pq[:h0, :])
                        # upper half: Q lands in slot (l+2)%5, K/V stay at l
                        nc.scalar.copy(out=qkv_bf[h0:P, (l + 2) % L, 0:INNER],
                                       in_=pq[h0:P, 0:INNER])
                        nc.scalar.copy(out=qkv_bf[h0:P, l, INNER:],
                                       in_=pq[h0:P, INNER:])
                    else:
                        nc.scalar.copy(out=qkv_bf[:P, l, :], in_=pq[:P, :])
                st["qkv_bf"] = qkv_bf

            def stage_a1(st):
                """QK^T on DVE; kicks off ACT exp."""
                P = st["P"]
                ni = 3 if st["rot"] else L
                st["ni"] = ni
                qkv_bf = st["qkv_bf"]
                # q[p, i, (m d)] bcast over j;  k[p, j, (m d)] bcast over i
                q_v = qkv_bf[:P, 0:ni, 0:INNER].unsqueeze(2).broadcast_to([P, ni, L, INNER])
                k_v = qkv_bf[:P, :, INNER:2 * INNER].unsqueeze(1).broadcast_to([P, ni, L, INNER])
                qk = peri.tile([128, L * L * HEADS, DIM_HEAD], bf16, tag="qk")
                qk = qk[:, 0:ni * L * HEADS, :]
                nc.vector.tensor_mul(
                    qk[:P].rearrange("p (i j m) d -> p i j (m d)", i=ni, j=L),
                    q_v, k_v,
                )
                # pairwise d-tree: 32 -> 16 -> 8 -> 4 -> 2 -> 1
                ns = ni * L * HEADS
                t16 = peri.tile([128, L * L * HEADS, 16], bf16, tag="t16")
                nc.vector.tensor_add(t16[:P, 0:ns], qk[:P, :, 0:16], qk[:P, :, 16:32])
                t8 = peri.tile([128, L * L * HEADS, 8], bf16, tag="t8")
                nc.vector.tensor_add(t8[:P, 0:ns], t16[:P, 0:ns, 0:8], t16[:P, 0:ns, 8:16])
                t4 = peri.tile([128, L * L * HEADS, 4], bf16, tag="t4")
                nc.vector.tensor_add(t4[:P, 0:ns], t8[:P, 0:ns, 0:4], t8[:P, 0:ns, 4:8])
                t2 = peri.tile([128, L * L * HEADS, 2], bf16, tag="t2")
                nc.vector.tensor_add(t2[:P, 0:ns], t4[:P, 0:ns, 0:2], t4[:P, 0:ns, 2:4])
                # final fold in f32 (exp-input precision)
                A = peri.tile([128, L, L, HEADS], f32, tag="A")
                A = A[:, 0:ni]
                nc.vector.tensor_add(
                    A[:P],
                    t2[:P, 0:ns, 0].rearrange("p (i j m) -> p i j m", i=ni, j=L),
                    t2[:P, 0:ns, 1].rearrange("p (i j m) -> p i j m", i=ni, j=L),
                )
                # exp on ACT (runs while DVE does the AV stage of tile t-2)
                eu = peri.tile([128, L, L, HEADS], bf16, tag="eu")
                eu = eu[:, 0:ni]
                nc.scalar.activation(
                    out=eu[:P], in_=A[:P], func=mybir.ActivationFunctionType.Exp,
                    scale=SCALE,
                )
                st["eu"] = eu

            def stage_a2(st):
                """softmax tail: mask, j-sum, reciprocal, normalize -> st['pw']."""
                P = st["P"]
                ni = st["ni"]
                # multiplicative {1,0} mask after exp (bf16 2x beats the f32
                # additive -1e4 logit bias, and halves the mask DMA); placed
                # here so the AV stage of tile t-2 hides the exp round-trip
                ee = work.tile([128, L, L, HEADS], bf16, tag="ee")
                ee = ee[:, 0:ni]
                nc.vector.tensor_mul(
                    ee[:P], st["eu"][:P],
                    st["mbias"][:P].unsqueeze(1).unsqueeze(3).broadcast_to([P, ni, L, HEADS]),
                )
                ssum = work.tile([128, L, HEADS], f32, tag="ssum")
                nc.vector.reduce_sum(
                    out=ssum[:P, 0:ni], in_=ee[:P].transpose([0, 1, 3, 2]),
                    axis=mybir.AxisListType.X
                )
                # ~51 ULP approx, ~5x faster than iterative reciprocal; ssum is
                # strictly positive and well inside the safe range
                sinv = work.tile([128, L, HEADS], f32, tag="sinv")
                nc.vector.reciprocal_approx_fast(out=sinv[:P, 0:ni], in_=ssum[:P, 0:ni])
                pw = work.tile([128, L, L, HEADS], bf16, tag="pw")
                pw = pw[:, 0:ni]
                nc.vector.tensor_mul(
                    pw[:P], ee[:P],
                    sinv[:P, 0:ni].unsqueeze(2).broadcast_to([P, ni, L, HEADS]),
                )
                st["pw"] = pw

            def stage_b_av(st, ia, ib):
                """attention-weighted V for i in [ia, ib): one 4-dim broadcast
                mul + 3-op j-tree.

                V rides in (d, m) order (host-permuted w_qkv columns) so the pw
                broadcast lands on a non-inner dim; w_out rows are host-permuted
                to match."""
                P = st["P"]
                ni = ib - ia
                pw = st["pw"]
                qkv_bf = st["qkv_bf"]
                # v[p, j, d, m] bcast over i;  pw[p, i, j, m] bcast over d
                v4 = (qkv_bf[:P, :, 2 * INNER:3 * INNER]
                      .rearrange("p j (d m) -> p j d m", m=HEADS)
                      .unsqueeze(1).broadcast_to([P, ni, L, DIM_HEAD, HEADS]))
                pw4 = pw[:P, ia:ib].unsqueeze(3).broadcast_to([P, ni, L, DIM_HEAD, HEADS])
                if "av" not in st:
                    st["av"] = peri.tile([128, L, L, INNER], bf16, tag="av", name="av")
                    st["s2"] = peri.tile([128, L, 2, INNER], bf16, tag="s2", name="s2")
                    st["s1"] = peri.tile([128, L, INNER], bf16, tag="s1", name="s1")
                    st["attout"] = work.tile([128, L, INNER], bf16, tag="attout", name="attout")
                av, s2, s1, attout = st["av"], st["s2"], st["s1"], st["attout"]
                nc.vector.tensor_mul(
                    av[:P, ia:ib].rearrange("p i j (d m) -> p i j d m", m=HEADS), v4, pw4,
                )
                # j-tree: 5 -> (2+2) -> 1 (+ leftover j=4)
                nc.vector.tensor_add(s2[:P, ia:ib], av[:P, ia:ib, 0:2], av[:P, ia:ib, 2:4])
                nc.vector.tensor_add(s1[:P, ia:ib], s2[:P, ia:ib, 0], s2[:P, ia:ib, 1])
                nc.vector.tensor_add(attout[:P, ia:ib], s1[:P, ia:ib], av[:P, ia:ib, 4])

            def stage_b_out(st, ia, ib, do_dma=True, do_transpose=True,
                            do_proj=True):
                """PE transposes + transposed out-projection + bf16 store, for
                i in [ia, ib)."""
                P = st["P"]
                attout = st["attout"]
                # transpose attout[:, i, cc*128:(cc+1)*128] -> pt[:, i, cc, :P]
                if "pt" not in st:
                    st["pt"] = ps_t.tile([128, L, 2, 128], bf16, tag="pst", name="pt")
                    st["aoti"] = peri.tile([128, L, 2, 128], bf16, tag="aoti", name="aoti")
                    st["osb"] = peri.tile([128, 2, L, 128], bf16, tag="osb", name="osb")
                pt, aoti, osb = st["pt"], st["aoti"], st["osb"]
                if do_transpose:
                    for i in range(ia, ib):
                        for cc in range(2):
                            nc.tensor.transpose(
                                pt[:, i, cc, :P],
                                attout[:P, i, cc * 128:(cc + 1) * 128],
                                ident[:P, :P],
                            )
                    nc.scalar.copy(out=aoti[:, ia:ib], in_=pt[:, ia:ib])
                if not do_proj:
                    return
                # out^T[c_chunk, (i, loc)] = sum_cc wout[cc, c_chunk]^T @ aoti[cc]
                # accumulation groups must stay inside one 2KB PSUM bank
                # (f32 col 512 == i 4), so split i-ranges at 4.
                igroups = [(a, b) for (a, b) in ((ia, min(ib, 4)), (max(ia, 4), ib)) if a < b]
                for o in range(2):
                    po = ps_o.tile([128, L, 128], f32, tag="pso")
                    for (i0, i1) in igroups:
                        for cc in range(2):
                            nc.tensor.matmul(
                                out=po[:, i0:i1, :P],
                                lhsT=wout_sb[:, cc, o * 128:(o + 1) * 128],
                                rhs=aoti[:, i0:i1, cc, :P],
                                start=(cc == 0),
                                stop=(cc == 1),
                            )
                    nc.scalar.copy(out=osb[:, o, ia:ib], in_=po[:, ia:ib])
                if not do_dma:
                    return
                if st["rot"]:
                    (p0, b, h, w0, w1) = st["pieces"][0]
                    n = w1 - w0
                    for o in range(2):
                        # lower partition half: slots ia:ib are real l ia:ib
                        nc.sync.dma_start(
                            out=outT_d[o, :, b, h, ia:ib, w0:w1],
                            in_=osb[:, o, ia:ib, 0:n],
                        )
                        # upper half: slot s (s < 2) is real l s+3; slot 2 is a
                        # discarded duplicate
                        ja, jb = ia, min(ib, 2)
                        if ja < jb:
                            nc.sync.dma_start(
                                out=outT_d[o, :, b, h, ja + 3:jb + 3, w0:w1],
                                in_=osb[:, o, ja:jb, n:2 * n],
                            )
                else:
                    for (p0, b, h, w0, w1) in st["pieces"]:
                        for o in range(2):
                            nc.sync.dma_start(
                                out=outT_d[o, :, b, h, ia:ib, w0:w1],
                                in_=osb[:, o, ia:ib, p0:p0 + (w1 - w0)],
                            )

            # 3-deep software pipeline: per iteration t issue
            #   load(t)+proj(t)    DMA + PE qkv + ACT cast
            #   stage_a1(t-1)      DVE qk -> ACT exp
            #   stage_b(t-2)       DVE av (covers the exp round-trip)
            #   stage_a2(t-1)      DVE softmax tail
            #   stage_b_out(t-2)   PE transpose + out-proj; ACT cast; DMA store
            sts = {}
            for t in range(NTILES + 2):
                if t < NTILES:
                    sts[t] = load(t)
                    proj(sts[t])
                    if t == 0:
                        setup_tail()
                if 0 <= t - 1 < NTILES:
                    stage_a1(sts[t - 1])
                if 0 <= t - 2 < NTILES:
                    if t - 2 == NTILES - 1:
                        # last tile: i-extent 3 (rotated duplicate layout) and
                        # per-i backend chunks so its PE/ACT/DMA tail overlaps
                        # the tail of the DVE work; the output DMAs are
                        # coarsened into one final batch (10 small late DMAs
                        # serialized ~3us of drain otherwise)
                        for i in range(3):
                            stage_b_av(sts[t - 2], i, i + 1)
                            stage_b_out(sts[t - 2], i, i + 1, do_dma=False,
                                        do_proj=False)
                        stage_b_out(sts[t - 2], 0, 3, do_transpose=False)
                    else:
                        stage_b_av(sts[t - 2], 0, 5)
                if 0 <= t - 1 < NTILES:
                    stage_a2(sts[t - 1])
                if 0 <= t - 2 < NTILES:
                    if t - 2 != NTILES - 1:
                        stage_b_out(sts[t - 2], 0, 5)
                    del sts[t - 2]
    nc.finalize()  # Bacc.compile(): legalize multi-wait instructions, alloc regs
    return nc


def get_nc():
    if "nc" not in _cached:
        _cached["nc"] = _build_bass()
    return _cached["nc"]


def make_in_maps(x, mask, w_qkv, w_out, b_out):
    """Host-side shard + repack: x is transposed to [cc, c, b, h, l, w] and
    cast to bf16; the mask becomes an f32 additive logit bias."""
    import ml_dtypes

    x = np.asarray(x, dtype=np.float32)
    mask = np.asarray(mask)
    w_qkv = np.ascontiguousarray(np.asarray(w_qkv), dtype=np.float32)
    w_out = np.ascontiguousarray(np.asarray(w_out), dtype=np.float32)

    # permute V's output columns (m,d)->(d,m) and w_out's rows to match, so
    # the device-side pw broadcast is never on the innermost dim
    wv = w_qkv[:, 2 * INNER:].reshape(C, HEADS, DIM_HEAD).transpose(0, 2, 1).reshape(C, INNER)
    w_qkv = np.ascontiguousarray(
        np.concatenate([w_qkv[:, :2 * INNER], wv], axis=1).astype(ml_dtypes.bfloat16)
    )
    w_out = np.ascontiguousarray(
        w_out.reshape(HEADS, DIM_HEAD, C).transpose(1, 0, 2).reshape(INNER, C)
        .astype(ml_dtypes.bfloat16)
    )

    # [B, L, H, W, C] -> [C, B, H, L, W] -> [2, 128, B, H, L, W] bf16
    xT = np.transpose(x, (4, 0, 2, 1, 3)).astype(ml_dtypes.bfloat16)
    xT = np.ascontiguousarray(xT.reshape(2, 128, B, H, L, W))
    # [B, H, W, 1, L] -> bf16 multiplicative mask [B, H, W, L]
    mb = np.ascontiguousarray(
        (mask[:, :, :, 0, :] != 0).astype(ml_dtypes.bfloat16)
    )

    in_maps = []
    for k in range(NCORES):
        h0, h1 = k * HP, (k + 1) * HP
        in_maps.append({
            "xT": np.ascontiguousarray(xT[:, :, :, h0:h1]),
            "mbias": np.ascontiguousarray(mb[:, h0:h1]),
            "w_qkv": w_qkv,
            "w_out": w_out,
        })
    return in_maps


def assemble_out(results, b_out):
    """Host-side unshard: out^T bf16 [2, 128, B, HP, L, W] per core ->
    full f32 [B, L, H, W, C] (+ b_out)."""
    outT = np.concatenate([r["outT"] for r in results], axis=3)  # [2,128,B,H,L,W]
    out = np.transpose(outT, (2, 4, 3, 5, 0, 1)).reshape(B, L, H, W, C)
    return out.astype(np.float32) + np.asarray(b_out, dtype=np.float32)


def kernel(x, mask, w_qkv, w_out, b_out):
    from concourse.bass_utils import run_bass_kernel_spmd

    nc = get_nc()
    in_maps = make_in_maps(x, mask, w_qkv, w_out, b_out)
    res = run_bass_kernel_spmd(nc, in_maps, core_ids=list(range(NCORES)))
    return assemble_out(res.results, b_out)


# revision 26
# speedup vs baseline: 1.0296x; 1.0002x over previous
"""CavAttention Trainium2 kernel (fused-DVE rewrite).

Computation (per spatial location (b,h,w), L=5 "cav" slots, 8 heads x 32 dim):
  qkv = x @ w_qkv ; att = softmax_j(mask * q_i.k_j / sqrt(d)) ; o = att @ v ; out = o @ w_out + b_out

Distribution: shard the H axis (48) across the 8 cores (6 each); weights replicated.

Per-core layout: locations (b,h,w) ride the 128 SBUF partitions; (l, head, d)
rides the free axis in bf16 (DVE 2x packed mode: 0.52 ns/elem vs 1.04 at 1x).
Measured DVE cost law: 0.52 ns/elem (2x) + ~150 ns/instruction, so the
attention core is emitted as ~14 big fused ops per 128-location tile instead
of ~52 small ones: one broadcast QK mul [p,i,j,(m d)], a 5-op pairwise d-tree,
ACT exp (interleaved with the AV stage of the previous tile to hide the ACT
round-trip), a multiplicative bf16 {1,0} mask, j-reduce, fast reciprocal,
softmax normalize, one 4-dim broadcast AV mul [p,i,j,d,m], and a 3-op j-tree.
The steady state is gapless on DVE (~14.5 us/tile, PE/ACT/DMA all inside it).

The output projection runs transposed: w_out chunks are the PE stationary and
the transposed attention output streams through, producing out^T (c-major) in
PSUM. That kills the per-i bias matmuls (b_out is added on the host) and the
f32 staging copies; out^T is cast to bf16 on ACT (halving output DMA traffic)
and the host transposes back / upcasts.
"""

import numpy as np

B, L, H, W, C = 2, 5, 48, 176, 256
HEADS, DIM_HEAD = 8, 32
INNER = HEADS * DIM_HEAD  # 256
SCALE = DIM_HEAD ** -0.5
NCORES = 8
HP = H // NCORES  # 6 h-planes per core
NBH = B * HP      # 12 (b,h) blocks per core
LOCS = NBH * W    # 2112 locations per core
PTILE = 128       # locations per tile
NTILES = (LOCS + PTILE - 1) // PTILE  # 17

_cached = {}


def _pieces(s, e):
    """Split flat loc range [s,e) into (p0, b, h, w0, w1) pieces within (b,h) blocks."""
    out = []
    cur = s
    while cur < e:
        bh = cur // W
        w0 = cur % W
        w1 = min(W, w0 + (e - cur))
        out.append((cur - s, bh // HP, bh % HP, w0, w1))
        cur += w1 - w0
    return out


def _build_bass():
    import concourse.bass as bass
    import concourse.bacc as bacc
    import concourse.tile as tile
    from concourse import mybir
    from concourse.masks import make_identity

    f32 = mybir.dt.float32
    bf16 = mybir.dt.bfloat16

    nc = bacc.Bacc()
    # x arrives pre-transposed and pre-cast on the host: [cc, c, b, h, l, w]
    xT_d = nc.dram_tensor("xT", [2, 128, B, HP, L, W], bf16, kind="ExternalInput")
    # mask arrives as a bf16 multiplicative mask (1 valid / 0 masked)
    mb_d = nc.dram_tensor("mbias", [B, HP, W, L], bf16, kind="ExternalInput")
    wqkv_d = nc.dram_tensor("w_qkv", [C, 3 * INNER], bf16, kind="ExternalInput")
    wout_d = nc.dram_tensor("w_out", [INNER, C], bf16, kind="ExternalInput")
    # out^T, bf16, pre-bias: element (o, c, b, h, l, w) = out[b, l, h, w, o*128+c]
    outT_d = nc.dram_tensor("outT", [2, 128, B, HP, L, W], bf16, kind="ExternalOutput")

    with tile.TileContext(nc) as tc:
        with (
            tc.tile_pool(name="singles", bufs=1) as singles,
            tc.tile_pool(name="work", bufs=3) as work,
            tc.tile_pool(name="peri", bufs=2) as peri,
            tc.tile_pool(name="ps_t", bufs=1, space="PSUM") as ps_t,
            tc.tile_pool(name="ps_qkv", bufs=2, space="PSUM") as ps_qkv,
            tc.tile_pool(name="ps_o", bufs=1, space="PSUM") as ps_o,
        ):
            # ---- constants.  Tensors touched by PE matmuls are produced by
            #      ONE engine (ACT): PE instructions carry a single
            #      semaphore wait (walrus S3_LW limit).
            #      Order: w_qkv DMA + cast first — it gates proj(0), the head
            #      of the pipeline-fill critical path. ----
            # clock warm-up: dummy DVE + PE work while the weight DMAs are in
            # flight, so the activity monitor upshifts the core clock before
            # the first real compute (early ops otherwise run ~60% slow)
            cw = singles.tile([128, 4096], bf16)
            nc.vector.memzero(cw[:, 0:4096])
            for _ in range(3):
                nc.vector.tensor_mul(cw[:, 0:2048], cw[:, 0:2048], cw[:, 2048:4096])
            # weights arrive bf16 from the host (they are used in bf16 anyway);
            # staged through an ACT copy so PE matmul operands keep a single
            # producing engine (walrus single-wait limit)
            wqkv_l = singles.tile([128, 2, 3 * INNER], bf16)
            wqkv_sb = singles.tile([128, 2, 3 * INNER], bf16)
            for cc in range(2):
                # per-cc DMA + copy: proj(0)'s cc=0 matmuls start while the
                # cc=1 half is still in flight
                nc.sync.dma_start(out=wqkv_l[:, cc, :], in_=wqkv_d[cc * 128:(cc + 1) * 128, :])
                nc.scalar.copy(out=wqkv_sb[:, cc], in_=wqkv_l[:, cc])
            ident_l = singles.tile([128, 128], f32)
            make_identity(nc, ident_l)  # gpsimd
            wout_l = singles.tile([128, 2, C], bf16)

            ident = singles.tile([128, 128], bf16)
            wout_sb = singles.tile([128, 2, C], bf16)

            def setup_tail():
                """ident/wout DMA + casts — needed first by stage_b_out(0) in
                iteration 2; emitted after proj(0) so they sit behind neither
                the fill-critical xt DMAs in the SP queue nor the qkv
                psum->sbuf copies in the ACT queue."""
                nc.sync.dma_start(
                    out=wout_l,
                    in_=wout_d[:, :].rearrange("(cc p) n -> p cc n", cc=2),
                )
                nc.scalar.copy(out=ident, in_=ident_l)
                nc.scalar.copy(out=wout_sb, in_=wout_l)

            def load(t):
                """DMA in (x already transposed+bf16 on host; partitions = C-chunk).

                The last tile holds only 64 locations; they are DUPLICATED into
                both partition halves and the upper half's Q block is l-rotated
                by 3 in proj(), so the attention stages can run with i-extent 3
                instead of 5 (partitions 0:64 cover i 0..2, 64:128 cover the
                real i 3..4; the third upper slot is a discarded duplicate)."""
                s = t * PTILE
                e = min(s + PTILE, LOCS)
                P = e - s
                pieces = _pieces(s, e)
                rot = (t == NTILES - 1)

                xt = work.tile([128, 2, L, 128], bf16, tag="xt")
                for (p0, b, h, w0, w1) in pieces:
                    for cc in range(2):
                        nc.sync.dma_start(
                            out=xt[:, cc, :, p0:p0 + (w1 - w0)],
                            in_=xT_d[cc, :, b, h, :, w0:w1],
                        )
                        if rot:
                            nc.sync.dma_start(
                                out=xt[:, cc, :, P + p0:P + p0 + (w1 - w0)],
                                in_=xT_d[cc, :, b, h, :, w0:w1],
                            )
                mbias = work.tile([128, L], bf16, tag="mbias")
                for (p0, b, h, w0, w1) in pieces:
                    nc.sync.dma_start(
                        out=mbias[p0:p0 + (w1 - w0), :],
                        in_=mb_d[b, h, w0:w1, :],
                    )
                    if rot:
                        nc.sync.dma_start(
                            out=mbias[P + p0:P + p0 + (w1 - w0), :],
                            in_=mb_d[b, h, w0:w1, :],
                        )
                return dict(P=(2 * P if rot else P), pieces=pieces, xt=xt,
                            mbias=mbias, rot=rot, P0=P)

            def proj(st):
                """qkv projection on PE + ACT psum->sbuf cast."""
                P = st["P"]
                xt = st["xt"]
                qkv_bf = work.tile([128, L, 3 * INNER], bf16, tag="qkv_bf")
                for l in range(L):
                    pq = ps_qkv.tile([128, 3 * INNER], f32, tag="psq")
                    for cc in range(2):
                        for (n0, n1) in ((0, 512), (512, 768)):
                            nc.tensor.matmul(
                                out=pq[:P, n0:n1],
                                lhsT=xt[:, cc, l, :P],
                                rhs=wqkv_sb[:, cc, n0:n1],
                                start=(cc == 0),
                                stop=(cc == 1),
                            )
                    if st["rot"]:
                        h0 = st["P0"]
                        nc.scalar.copy(out=qkv_bf[:h0, l, :], in_=pq[:h0, :])
                        # upper half: Q lands in slot (l+2)%5, K/V stay at l
                        nc.scalar.copy(out=qkv_bf[h0:P, (l + 2) % L, 0:INNER],
                                       in_=pq[h0:P, 0:INNER])
                        nc.scalar.copy(out=qkv_bf[h0:P, l, INNER:],
                                       in_=pq[h0:P, INNER:])
                    else:
                        nc.scalar.copy(out=qkv_bf[:P, l, :], in_=pq[:P, :])
                st["qkv_bf"] = qkv_bf

            def stage_a1(st):
                """QK^T on DVE; kicks off ACT exp."""
                P = st["P"]
                ni = 3 if st["rot"] else L
                st["ni"] = ni
                qkv_bf = st["qkv_bf"]
                # q[p, i, (m d)] bcast over j;  k[p, j, (m d)] bcast over i
                q_v = qkv_bf[:P, 0:ni, 0:INNER].unsqueeze(2).broadcast_to([P, ni, L, INNER])
                k_v = qkv_bf[:P, :, INNER:2 * INNER].unsqueeze(1).broadcast_to([P, ni, L, INNER])
                qk = peri.tile([128, L * L * HEADS, DIM_HEAD], bf16, tag="qk")
                qk = qk[:, 0:ni * L * HEADS, :]
                nc.vector.tensor_mul(
                    qk[:P].rearrange("p (i j m) d -> p i j (m d)", i=ni, j=L),
                    q_v, k_v,
                )
                # pairwise d-tree: 32 -> 16 -> 8 -> 4 -> 2 -> 1
                ns = ni * L * HEADS
                t16 = peri.tile([128, L * L * HEADS, 16], bf16, tag="t16")
                nc.vector.tensor_add(t16[:P, 0:ns], qk[:P, :, 0:16], qk[:P, :, 16:32])
                t8 = peri.tile([128, L * L * HEADS, 8], bf16, tag="t8")
                nc.vector.tensor_add(t8[:P, 0:ns], t16[:P, 0:ns, 0:8], t16[:P, 0:ns, 8:16])
                t4 = peri.tile([128, L * L * HEADS, 4], bf16, tag="t4")
                nc.vector.tensor_add(t4[:P, 0:ns], t8[:P, 0:ns, 0:4], t8[:P, 0:ns, 4:8])
                t2 = peri.tile([128, L * L * HEADS, 2], bf16, tag="t2")
                nc.vector.tensor_add(t2[:P, 0:ns], t4[:P, 0:ns, 0:2], t4[:P, 0:ns, 2:4])
                # final fold in f32 (exp-input precision)
                A = peri.tile([128, L, L, HEADS], f32, tag="A")
                A = A[:, 0:ni]
                nc.vector.tensor_add(
                    A[:P],
                    t2[:P, 0:ns, 0].rearrange("p (i j m) -> p i j m", i=ni, j=L),
                    t2[:P, 0:ns, 1].rearrange("p (i j m) -> p i j m", i=ni, j=L),
                )
                # exp on ACT (runs while DVE does the AV stage of tile t-2)
                eu = peri.tile([128, L, L, HEADS], bf16, tag="eu")
                eu = eu[:, 0:ni]
                nc.scalar.activation(
                    out=eu[:P], in_=A[:P], func=mybir.ActivationFunctionType.Exp,
                    scale=SCALE,
                )
                st["eu"] = eu

            def stage_a2(st):
                """softmax tail: mask, j-sum, reciprocal, normalize -> st['pw']."""
                P = st["P"]
                ni = st["ni"]
                # multiplicative {1,0} mask after exp (bf16 2x beats the f32
                # additive -1e4 logit bias, and halves the mask DMA); placed
                # here so the AV stage of tile t-2 hides the exp round-trip
                ee = work.tile([128, L, L, HEADS], bf16, tag="ee")
                ee = ee[:, 0:ni]
                nc.vector.tensor_mul(
                    ee[:P], st["eu"][:P],
                    st["mbias"][:P].unsqueeze(1).unsqueeze(3).broadcast_to([P, ni, L, HEADS]),
                )
                ssum = work.tile([128, L, HEADS], f32, tag="ssum")
                nc.vector.reduce_sum(
                    out=ssum[:P, 0:ni], in_=ee[:P].transpose([0, 1, 3, 2]),
                    axis=mybir.AxisListType.X
                )
                # ~51 ULP approx, ~5x faster than iterative reciprocal; ssum is
                # strictly positive and well inside the safe range
                sinv = work.tile([128, L, HEADS], f32, tag="sinv")
                nc.vector.reciprocal_approx_fast(out=sinv[:P, 0:ni], in_=ssum[:P, 0:ni])
                pw = work.tile([128, L, L, HEADS], bf16, tag="pw")
                pw = pw[:, 0:ni]
                nc.vector.tensor_mul(
                    pw[:P], ee[:P],
                    sinv[:P, 0:ni].unsqueeze(2).broadcast_to([P, ni, L, HEADS]),
                )
                st["pw"] = pw

            def stage_b_av(st, ia, ib):
                """attention-weighted V for i in [ia, ib): one 4-dim broadcast
                mul + 3-op j-tree.

                V rides in (d, m) order (host-permuted w_qkv columns) so the pw
                broadcast lands on a non-inner dim; w_out rows are host-permuted
                to match."""
                P = st["P"]
                ni = ib - ia
                pw = st["pw"]
                qkv_bf = st["qkv_bf"]
                # v[p, j, d, m] bcast over i;  pw[p, i, j, m] bcast over d
                v4 = (qkv_bf[:P, :, 2 * INNER:3 * INNER]
                      .rearrange("p j (d m) -> p j d m", m=HEADS)
                      .unsqueeze(1).broadcast_to([P, ni, L, DIM_HEAD, HEADS]))
                pw4 = pw[:P, ia:ib].unsqueeze(3).broadcast_to([P, ni, L, DIM_HEAD, HEADS])
                if "av" not in st:
                    st["av"] = peri.tile([128, L, L, INNER], bf16, tag="av", name="av")
                    st["s2"] = peri.tile([128, L, 2, INNER], bf16, tag="s2", name="s2")
                    st["s1"] = peri.tile([128, L, INNER], bf16, tag="s1", name="s1")
                    st["attout"] = work.tile([128, L, INNER], bf16, tag="attout", name="attout")
                av, s2, s1, attout = st["av"], st["s2"], st["s1"], st["attout"]
                nc.vector.tensor_mul(
                    av[:P, ia:ib].rearrange("p i j (d m) -> p i j d m", m=HEADS), v4, pw4,
                )
                # j-tree: 5 -> (2+2) -> 1 (+ leftover j=4)
                nc.vector.tensor_add(s2[:P, ia:ib], av[:P, ia:ib, 0:2], av[:P, ia:ib, 2:4])
                nc.vector.tensor_add(s1[:P, ia:ib], s2[:P, ia:ib, 0], s2[:P, ia:ib, 1])
                nc.vector.tensor_add(attout[:P, ia:ib], s1[:P, ia:ib], av[:P, ia:ib, 4])

            def stage_b_out(st, ia, ib, do_dma=True, do_transpose=True,
                            do_proj=True):
                """PE transposes + transposed out-projection + bf16 store, for
                i in [ia, ib)."""
                P = st["P"]
                attout = st["attout"]
                # transpose attout[:, i, cc*128:(cc+1)*128] -> pt[:, i, cc, :P]
                if "pt" not in st:
                    st["pt"] = ps_t.tile([128, L, 2, 128], bf16, tag="pst", name="pt")
                    st["aoti"] = peri.tile([128, L, 2, 128], bf16, tag="aoti", name="aoti")
                    st["osb"] = peri.tile([128, 2, L, 128], bf16, tag="osb", name="osb")
                pt, aoti, osb = st["pt"], st["aoti"], st["osb"]
                if do_transpose:
                    for i in range(ia, ib):
                        for cc in range(2):
                            nc.tensor.transpose(
                                pt[:, i, cc, :P],
                                attout[:P, i, cc * 128:(cc + 1) * 128],
                                ident[:P, :P],
                            )
                    nc.scalar.copy(out=aoti[:, ia:ib], in_=pt[:, ia:ib])
                if not do_proj:
                    return
                # out^T[c_chunk, (i, loc)] = sum_cc wout[cc, c_chunk]^T @ aoti[cc]
                # accumulation groups must stay inside one 2KB PSUM bank
                # (f32 col 512 == i 4), so split i-ranges at 4.
                if st["rot"]:
                    # both c-chunks in one padded allocation (same 4KB tag
                    # footprint; o=1 starts at byte 2048 so each acc group
                    # stays inside a PSUM bank) — avoids serializing o=1's
                    # matmuls behind o=0's ACT copy in the drain
                    po2 = ps_o.tile([128, 2, 4, 128], f32, tag="pso", name="po2")
                    for o in range(2):
                        for cc in range(2):
                            nc.tensor.matmul(
                                out=po2[:, o, ia:ib, :P],
                                lhsT=wout_sb[:, cc, o * 128:(o + 1) * 128],
                                rhs=aoti[:, ia:ib, cc, :P],
                                start=(cc == 0),
                                stop=(cc == 1),
                            )
                    nc.scalar.copy(out=osb[:, :, ia:ib], in_=po2[:, :, ia:ib])
                else:
                    igroups = [(a, b) for (a, b) in ((ia, min(ib, 4)), (max(ia, 4), ib)) if a < b]
                    for o in range(2):
                        po = ps_o.tile([128, L, 128], f32, tag="pso")
                        for (i0, i1) in igroups:
                            for cc in range(2):
                                nc.tensor.matmul(
                                    out=po[:, i0:i1, :P],
                                    lhsT=wout_sb[:, cc, o * 128:(o + 1) * 128],
                                    rhs=aoti[:, i0:i1, cc, :P],
                                    start=(cc == 0),
                                    stop=(cc == 1),
                                )
                        nc.scalar.copy(out=osb[:, o, ia:ib], in_=po[:, ia:ib])
                if not do_dma:
                    return
                if st["rot"]:
                    (p0, b, h, w0, w1) = st["pieces"][0]
                    n = w1 - w0
                    for o in range(2):
                        # lower partition half: slots ia:ib are real l ia:ib
                        nc.sync.dma_start(
                            out=outT_d[o, :, b, h, ia:ib, w0:w1],
                            in_=osb[:, o, ia:ib, 0:n],
                        )
                        # upper half: slot s (s < 2) is real l s+3; slot 2 is a
                        # discarded duplicate
                        ja, jb = ia, min(ib, 2)
                        if ja < jb:
                            nc.sync.dma_start(
                                out=outT_d[o, :, b, h, ja + 3:jb + 3, w0:w1],
                                in_=osb[:, o, ja:jb, n:2 * n],
                            )
                else:
                    for (p0, b, h, w0, w1) in st["pieces"]:
                        for o in range(2):
                            nc.sync.dma_start(
                                out=outT_d[o, :, b, h, ia:ib, w0:w1],
                                in_=osb[:, o, ia:ib, p0:p0 + (w1 - w0)],
                            )

            # 3-deep software pipeline: per iteration t issue
            #   load(t)+proj(t)    DMA + PE qkv + ACT cast
            #   stage_a1(t-1)      DVE qk -> ACT exp
            #   stage_b(t-2)       DVE av (covers the exp round-trip)
            #   stage_a2(t-1)      DVE softmax tail
            #   stage_b_out(t-2)   PE transpose + out-proj; ACT cast; DMA store
            sts = {}
            for t in range(NTILES + 2):
                if t < NTILES:
                    sts[t] = load(t)
                    proj(sts[t])
                    if t == 0:
                        setup_tail()
                if 0 <= t - 1 < NTILES:
                    stage_a1(sts[t - 1])
                if 0 <= t - 2 < NTILES:
                    if t - 2 == NTILES - 1:
                        # last tile: i-extent 3 (rotated duplicate layout) and
                        # per-i backend chunks so its PE/ACT/DMA tail overlaps
                        # the tail of the DVE work; the output DMAs are
                        # coarsened into one final batch (10 small late DMAs
                        # serialized ~3us of drain otherwise)
                        for i in range(3):
                            stage_b_av(sts[t - 2], i, i + 1)
                            stage_b_out(sts[t - 2], i, i + 1, do_dma=False,
                                        do_proj=False)
                        stage_b_out(sts[t - 2], 0, 3, do_transpose=False)
                    else:
                        stage_b_av(sts[t - 2], 0, 5)
                if 0 <= t - 1 < NTILES:
                    stage_a2(sts[t - 1])
                if 0 <= t - 2 < NTILES:
                    if t - 2 != NTILES - 1:
                        stage_b_out(sts[t - 2], 0, 5)
                    del sts[t - 2]
    nc.finalize()  # Bacc.compile(): legalize multi-wait instructions, alloc regs
    return nc


def get_nc():
    if "nc" not in _cached:
        _cached["nc"] = _build_bass()
    return _cached["nc"]


def make_in_maps(x, mask, w_qkv, w_out, b_out):
    """Host-side shard + repack: x is transposed to [cc, c, b, h, l, w] and
    cast to bf16; the mask becomes an f32 additive logit bias."""
    import ml_dtypes

    x = np.asarray(x, dtype=np.float32)
    mask = np.asarray(mask)
    w_qkv = np.ascontiguousarray(np.asarray(w_qkv), dtype=np.float32)
    w_out = np.ascontiguousarray(np.asarray(w_out), dtype=np.float32)

    # permute V's output columns (m,d)->(d,m) and w_out's rows to match, so
    # the device-side pw broadcast is never on the innermost dim
    wv = w_qkv[:, 2 * INNER:].reshape(C, HEADS, DIM_HEAD).transpose(0, 2, 1).reshape(C, INNER)
    w_qkv = np.ascontiguousarray(
        np.concatenate([w_qkv[:, :2 * INNER], wv], axis=1).astype(ml_dtypes.bfloat16)
    )
    w_out = np.ascontiguousarray(
        w_out.reshape(HEADS, DIM_HEAD, C).transpose(1, 0, 2).reshape(INNER, C)
        .astype(ml_dtypes.bfloat16)
    )

    # [B, L, H, W, C] -> [C, B, H, L, W] -> [2, 128, B, H, L, W] bf16
    xT = np.transpose(x, (4, 0, 2, 1, 3)).astype(ml_dtypes.bfloat16)
    xT = np.ascontiguousarray(xT.reshape(2, 128, B, H, L, W))
    # [B, H, W, 1, L] -> bf16 multiplicative mask [B, H, W, L]
    mb = np.ascontiguousarray(
        (mask[:, :, :, 0, :] != 0).astype(ml_dtypes.bfloat16)
    )

    in_maps = []
    for k in range(NCORES):
        h0, h1 = k * HP, (k + 1) * HP
        in_maps.append({
            "xT": np.ascontiguousarray(xT[:, :, :, h0:h1]),
            "mbias": np.ascontiguousarray(mb[:, h0:h1]),
            "w_qkv": w_qkv,
            "w_out": w_out,
        })
    return in_maps


def assemble_out(results, b_out):
    """Host-side unshard: out^T bf16 [2, 128, B, HP, L, W] per core ->
    full f32 [B, L, H, W, C] (+ b_out)."""
    outT = np.concatenate([r["outT"] for r in results], axis=3)  # [2,128,B,H,L,W]
    out = np.transpose(outT, (2, 4, 3, 5, 0, 1)).reshape(B, L, H, W, C)
    return out.astype(np.float32) + np.asarray(b_out, dtype=np.float32)


def kernel(x, mask, w_qkv, w_out, b_out):
    from concourse.bass_utils import run_bass_kernel_spmd

    nc = get_nc()
    in_maps = make_in_maps(x, mask, w_qkv, w_out, b_out)
    res = run_bass_kernel_spmd(nc, in_maps, core_ids=list(range(NCORES)))
    return assemble_out(res.results, b_out)


# revision 27
# speedup vs baseline: 1.0306x; 1.0009x over previous
"""CavAttention Trainium2 kernel (fused-DVE rewrite).

Computation (per spatial location (b,h,w), L=5 "cav" slots, 8 heads x 32 dim):
  qkv = x @ w_qkv ; att = softmax_j(mask * q_i.k_j / sqrt(d)) ; o = att @ v ; out = o @ w_out + b_out

Distribution: shard the H axis (48) across the 8 cores (6 each); weights replicated.

Per-core layout: locations (b,h,w) ride the 128 SBUF partitions; (l, head, d)
rides the free axis in bf16 (DVE 2x packed mode: 0.52 ns/elem vs 1.04 at 1x).
Measured DVE cost law: 0.52 ns/elem (2x) + ~150 ns/instruction, so the
attention core is emitted as ~14 big fused ops per 128-location tile instead
of ~52 small ones: one broadcast QK mul [p,i,j,(m d)], a 5-op pairwise d-tree,
ACT exp (interleaved with the AV stage of the previous tile to hide the ACT
round-trip), a multiplicative bf16 {1,0} mask, j-reduce, fast reciprocal,
softmax normalize, one 4-dim broadcast AV mul [p,i,j,d,m], and a 3-op j-tree.
The steady state is gapless on DVE (~14.5 us/tile, PE/ACT/DMA all inside it).

The output projection runs transposed: w_out chunks are the PE stationary and
the transposed attention output streams through, producing out^T (c-major) in
PSUM. That kills the per-i bias matmuls (b_out is added on the host) and the
f32 staging copies; out^T is cast to bf16 on ACT (halving output DMA traffic)
and the host transposes back / upcasts.
"""

import numpy as np

B, L, H, W, C = 2, 5, 48, 176, 256
HEADS, DIM_HEAD = 8, 32
INNER = HEADS * DIM_HEAD  # 256
SCALE = DIM_HEAD ** -0.5
NCORES = 8
HP = H // NCORES  # 6 h-planes per core
NBH = B * HP      # 12 (b,h) blocks per core
LOCS = NBH * W    # 2112 locations per core
PTILE = 128       # locations per tile
NTILES = (LOCS + PTILE - 1) // PTILE  # 17

_cached = {}


def _pieces(s, e):
    """Split flat loc range [s,e) into (p0, b, h, w0, w1) pieces within (b,h) blocks."""
    out = []
    cur = s
    while cur < e:
        bh = cur // W
        w0 = cur % W
        w1 = min(W, w0 + (e - cur))
        out.append((cur - s, bh // HP, bh % HP, w0, w1))
        cur += w1 - w0
    return out


def _build_bass():
    import concourse.bass as bass
    import concourse.bacc as bacc
    import concourse.tile as tile
    from concourse import mybir
    from concourse.masks import make_identity

    f32 = mybir.dt.float32
    bf16 = mybir.dt.bfloat16

    nc = bacc.Bacc()
    # x arrives pre-transposed and pre-cast on the host: [cc, c, b, h, l, w]
    xT_d = nc.dram_tensor("xT", [2, 128, B, HP, L, W], bf16, kind="ExternalInput")
    # mask arrives as a bf16 multiplicative mask (1 valid / 0 masked)
    mb_d = nc.dram_tensor("mbias", [B, HP, W, L], bf16, kind="ExternalInput")
    wqkv_d = nc.dram_tensor("w_qkv", [C, 3 * INNER], bf16, kind="ExternalInput")
    wout_d = nc.dram_tensor("w_out", [INNER, C], bf16, kind="ExternalInput")
    # out^T, bf16, pre-bias: element (o, c, b, h, l, w) = out[b, l, h, w, o*128+c]
    outT_d = nc.dram_tensor("outT", [2, 128, B, HP, L, W], bf16, kind="ExternalOutput")

    with tile.TileContext(nc) as tc:
        with (
            tc.tile_pool(name="singles", bufs=1) as singles,
            tc.tile_pool(name="work", bufs=3) as work,
            tc.tile_pool(name="peri", bufs=2) as peri,
            tc.tile_pool(name="ps_t", bufs=1, space="PSUM") as ps_t,
            tc.tile_pool(name="ps_qkv", bufs=2, space="PSUM") as ps_qkv,
            tc.tile_pool(name="ps_o", bufs=1, space="PSUM") as ps_o,
        ):
            # ---- constants.  Tensors touched by PE matmuls are produced by
            #      ONE engine (ACT): PE instructions carry a single
            #      semaphore wait (walrus S3_LW limit).
            #      Order: w_qkv DMA + cast first — it gates proj(0), the head
            #      of the pipeline-fill critical path. ----
            # clock warm-up: dummy DVE + PE work while the weight DMAs are in
            # flight, so the activity monitor upshifts the core clock before
            # the first real compute (early ops otherwise run ~60% slow)
            cw = singles.tile([128, 4096], bf16)
            nc.vector.memzero(cw[:, 0:4096])
            for _ in range(3):
                nc.vector.tensor_mul(cw[:, 0:2048], cw[:, 0:2048], cw[:, 2048:4096])
            # weights arrive bf16 from the host (they are used in bf16 anyway);
            # staged through an ACT copy so PE matmul operands keep a single
            # producing engine (walrus single-wait limit)
            wqkv_l = singles.tile([128, 2, 3 * INNER], bf16)
            wqkv_sb = singles.tile([128, 2, 3 * INNER], bf16)
            for cc in range(2):
                # per-cc DMA + copy: proj(0)'s cc=0 matmuls start while the
                # cc=1 half is still in flight
                nc.sync.dma_start(out=wqkv_l[:, cc, :], in_=wqkv_d[cc * 128:(cc + 1) * 128, :])
                nc.scalar.copy(out=wqkv_sb[:, cc], in_=wqkv_l[:, cc])
            ident_l = singles.tile([128, 128], f32)
            wout_l = singles.tile([128, 2, C], bf16)

            ident = singles.tile([128, 128], bf16)
            wout_sb = singles.tile([128, 2, C], bf16)

            def setup_tail():
                """ident build + wout DMA + casts — needed first by
                stage_b_out(0) in iteration 2; emitted after proj(0) so they
                sit behind neither the fill-critical xt DMAs in the SP queue
                nor the wqkv/qkv copies in the ACT queue."""
                make_identity(nc, ident_l)  # gpsimd
                nc.sync.dma_start(
                    out=wout_l,
                    in_=wout_d[:, :].rearrange("(cc p) n -> p cc n", cc=2),
                )
                nc.scalar.copy(out=ident, in_=ident_l)
                nc.scalar.copy(out=wout_sb, in_=wout_l)

            def load(t):
                """DMA in (x already transposed+bf16 on host; partitions = C-chunk).

                The last tile holds only 64 locations; they are DUPLICATED into
                both partition halves and the upper half's Q block is l-rotated
                by 3 in proj(), so the attention stages can run with i-extent 3
                instead of 5 (partitions 0:64 cover i 0..2, 64:128 cover the
                real i 3..4; the third upper slot is a discarded duplicate)."""
                s = t * PTILE
                e = min(s + PTILE, LOCS)
                P = e - s
                pieces = _pieces(s, e)
                rot = (t == NTILES - 1)

                xt = work.tile([128, 2, L, 128], bf16, tag="xt")
                for (p0, b, h, w0, w1) in pieces:
                    for cc in range(2):
                        nc.sync.dma_start(
                            out=xt[:, cc, :, p0:p0 + (w1 - w0)],
                            in_=xT_d[cc, :, b, h, :, w0:w1],
                        )
                        if rot:
                            nc.sync.dma_start(
                                out=xt[:, cc, :, P + p0:P + p0 + (w1 - w0)],
                                in_=xT_d[cc, :, b, h, :, w0:w1],
                            )
                mbias = work.tile([128, L], bf16, tag="mbias")
                for (p0, b, h, w0, w1) in pieces:
                    nc.sync.dma_start(
                        out=mbias[p0:p0 + (w1 - w0), :],
                        in_=mb_d[b, h, w0:w1, :],
                    )
                    if rot:
                        nc.sync.dma_start(
                            out=mbias[P + p0:P + p0 + (w1 - w0), :],
                            in_=mb_d[b, h, w0:w1, :],
                        )
                return dict(P=(2 * P if rot else P), pieces=pieces, xt=xt,
                            mbias=mbias, rot=rot, P0=P)

            def proj(st):
                """qkv projection on PE + ACT psum->sbuf cast."""
                P = st["P"]
                xt = st["xt"]
                qkv_bf = work.tile([128, L, 3 * INNER], bf16, tag="qkv_bf")
                for l in range(L):
                    pq = ps_qkv.tile([128, 3 * INNER], f32, tag="psq")
                    for cc in range(2):
                        for (n0, n1) in ((0, 512), (512, 768)):
                            nc.tensor.matmul(
                                out=pq[:P, n0:n1],
                                lhsT=xt[:, cc, l, :P],
                                rhs=wqkv_sb[:, cc, n0:n1],
                                start=(cc == 0),
                                stop=(cc == 1),
                            )
                    if st["rot"]:
                        h0 = st["P0"]
                        nc.scalar.copy(out=qkv_bf[:h0, l, :], in_=pq[:h0, :])
                        # upper half: Q lands in slot (l+2)%5, K/V stay at l
                        nc.scalar.copy(out=qkv_bf[h0:P, (l + 2) % L, 0:INNER],
                                       in_=pq[h0:P, 0:INNER])
                        nc.scalar.copy(out=qkv_bf[h0:P, l, INNER:],
                                       in_=pq[h0:P, INNER:])
                    else:
                        nc.scalar.copy(out=qkv_bf[:P, l, :], in_=pq[:P, :])
                st["qkv_bf"] = qkv_bf

            def stage_a1(st):
                """QK^T on DVE; kicks off ACT exp."""
                P = st["P"]
                ni = 3 if st["rot"] else L
                st["ni"] = ni
                qkv_bf = st["qkv_bf"]
                # q[p, i, (m d)] bcast over j;  k[p, j, (m d)] bcast over i
                q_v = qkv_bf[:P, 0:ni, 0:INNER].unsqueeze(2).broadcast_to([P, ni, L, INNER])
                k_v = qkv_bf[:P, :, INNER:2 * INNER].unsqueeze(1).broadcast_to([P, ni, L, INNER])
                qk = peri.tile([128, L * L * HEADS, DIM_HEAD], bf16, tag="qk")
                qk = qk[:, 0:ni * L * HEADS, :]
                nc.vector.tensor_mul(
                    qk[:P].rearrange("p (i j m) d -> p i j (m d)", i=ni, j=L),
                    q_v, k_v,
                )
                # pairwise d-tree: 32 -> 16 -> 8 -> 4 -> 2 -> 1
                ns = ni * L * HEADS
                t16 = peri.tile([128, L * L * HEADS, 16], bf16, tag="t16")
                nc.vector.tensor_add(t16[:P, 0:ns], qk[:P, :, 0:16], qk[:P, :, 16:32])
                t8 = peri.tile([128, L * L * HEADS, 8], bf16, tag="t8")
                nc.vector.tensor_add(t8[:P, 0:ns], t16[:P, 0:ns, 0:8], t16[:P, 0:ns, 8:16])
                t4 = peri.tile([128, L * L * HEADS, 4], bf16, tag="t4")
                nc.vector.tensor_add(t4[:P, 0:ns], t8[:P, 0:ns, 0:4], t8[:P, 0:ns, 4:8])
                t2 = peri.tile([128, L * L * HEADS, 2], bf16, tag="t2")
                nc.vector.tensor_add(t2[:P, 0:ns], t4[:P, 0:ns, 0:2], t4[:P, 0:ns, 2:4])
                # final fold in f32 (exp-input precision)
                A = peri.tile([128, L, L, HEADS], f32, tag="A")
                A = A[:, 0:ni]
                nc.vector.tensor_add(
                    A[:P],
                    t2[:P, 0:ns, 0].rearrange("p (i j m) -> p i j m", i=ni, j=L),
                    t2[:P, 0:ns, 1].rearrange("p (i j m) -> p i j m", i=ni, j=L),
                )
                # exp on ACT (runs while DVE does the AV stage of tile t-2)
                eu = peri.tile([128, L, L, HEADS], bf16, tag="eu")
                eu = eu[:, 0:ni]
                nc.scalar.activation(
                    out=eu[:P], in_=A[:P], func=mybir.ActivationFunctionType.Exp,
                    scale=SCALE,
                )
                st["eu"] = eu

            def stage_a2(st):
                """softmax tail: mask, j-sum, reciprocal, normalize -> st['pw']."""
                P = st["P"]
                ni = st["ni"]
                # multiplicative {1,0} mask after exp (bf16 2x beats the f32
                # additive -1e4 logit bias, and halves the mask DMA); placed
                # here so the AV stage of tile t-2 hides the exp round-trip
                ee = work.tile([128, L, L, HEADS], bf16, tag="ee")
                ee = ee[:, 0:ni]
                nc.vector.tensor_mul(
                    ee[:P], st["eu"][:P],
                    st["mbias"][:P].unsqueeze(1).unsqueeze(3).broadcast_to([P, ni, L, HEADS]),
                )
                ssum = work.tile([128, L, HEADS], f32, tag="ssum")
                nc.vector.reduce_sum(
                    out=ssum[:P, 0:ni], in_=ee[:P].transpose([0, 1, 3, 2]),
                    axis=mybir.AxisListType.X
                )
                # ~51 ULP approx, ~5x faster than iterative reciprocal; ssum is
                # strictly positive and well inside the safe range
                sinv = work.tile([128, L, HEADS], f32, tag="sinv")
                nc.vector.reciprocal_approx_fast(out=sinv[:P, 0:ni], in_=ssum[:P, 0:ni])
                pw = work.tile([128, L, L, HEADS], bf16, tag="pw")
                pw = pw[:, 0:ni]
                nc.vector.tensor_mul(
                    pw[:P], ee[:P],
                    sinv[:P, 0:ni].unsqueeze(2).broadcast_to([P, ni, L, HEADS]),
                )
                st["pw"] = pw

            def stage_b_av(st, ia, ib):
                """attention-weighted V for i in [ia, ib): one 4-dim broadcast
                mul + 3-op j-tree.

                V rides in (d, m) order (host-permuted w_qkv columns) so the pw
                broadcast lands on a non-inner dim; w_out rows are host-permuted
                to match."""
                P = st["P"]
                ni = ib - ia
                pw = st["pw"]
                qkv_bf = st["qkv_bf"]
                # v[p, j, d, m] bcast over i;  pw[p, i, j, m] bcast over d
                v4 = (qkv_bf[:P, :, 2 * INNER:3 * INNER]
                      .rearrange("p j (d m) -> p j d m", m=HEADS)
                      .unsqueeze(1).broadcast_to([P, ni, L, DIM_HEAD, HEADS]))
                pw4 = pw[:P, ia:ib].unsqueeze(3).broadcast_to([P, ni, L, DIM_HEAD, HEADS])
                if "av" not in st:
                    st["av"] = peri.tile([128, L, L, INNER], bf16, tag="av", name="av")
                    st["s2"] = peri.tile([128, L, 2, INNER], bf16, tag="s2", name="s2")
                    st["s1"] = peri.tile([128, L, INNER], bf16, tag="s1", name="s1")
                    st["attout"] = work.tile([128, L, INNER], bf16, tag="attout", name="attout")
                av, s2, s1, attout = st["av"], st["s2"], st["s1"], st["attout"]
                nc.vector.tensor_mul(
                    av[:P, ia:ib].rearrange("p i j (d m) -> p i j d m", m=HEADS), v4, pw4,
                )
                # j-tree: 5 -> (2+2) -> 1 (+ leftover j=4)
                nc.vector.tensor_add(s2[:P, ia:ib], av[:P, ia:ib, 0:2], av[:P, ia:ib, 2:4])
                nc.vector.tensor_add(s1[:P, ia:ib], s2[:P, ia:ib, 0], s2[:P, ia:ib, 1])
                nc.vector.tensor_add(attout[:P, ia:ib], s1[:P, ia:ib], av[:P, ia:ib, 4])

            def stage_b_out(st, ia, ib, do_dma=True, do_transpose=True,
                            do_proj=True):
                """PE transposes + transposed out-projection + bf16 store, for
                i in [ia, ib)."""
                P = st["P"]
                attout = st["attout"]
                # transpose attout[:, i, cc*128:(cc+1)*128] -> pt[:, i, cc, :P]
                if "pt" not in st:
                    st["pt"] = ps_t.tile([128, L, 2, 128], bf16, tag="pst", name="pt")
                    st["aoti"] = peri.tile([128, L, 2, 128], bf16, tag="aoti", name="aoti")
                    st["osb"] = peri.tile([128, 2, L, 128], bf16, tag="osb", name="osb")
                pt, aoti, osb = st["pt"], st["aoti"], st["osb"]
                if do_transpose:
                    for i in range(ia, ib):
                        for cc in range(2):
                            nc.tensor.transpose(
                                pt[:, i, cc, :P],
                                attout[:P, i, cc * 128:(cc + 1) * 128],
                                ident[:P, :P],
                            )
                    nc.scalar.copy(out=aoti[:, ia:ib], in_=pt[:, ia:ib])
                if not do_proj:
                    return
                # out^T[c_chunk, (i, loc)] = sum_cc wout[cc, c_chunk]^T @ aoti[cc]
                # accumulation groups must stay inside one 2KB PSUM bank
                # (f32 col 512 == i 4), so split i-ranges at 4.
                if st["rot"]:
                    # both c-chunks in one padded allocation (same 4KB tag
                    # footprint; o=1 starts at byte 2048 so each acc group
                    # stays inside a PSUM bank) — avoids serializing o=1's
                    # matmuls behind o=0's ACT copy in the drain
                    po2 = ps_o.tile([128, 2, 4, 128], f32, tag="pso", name="po2")
                    for o in range(2):
                        for cc in range(2):
                            nc.tensor.matmul(
                                out=po2[:, o, ia:ib, :P],
                                lhsT=wout_sb[:, cc, o * 128:(o + 1) * 128],
                                rhs=aoti[:, ia:ib, cc, :P],
                                start=(cc == 0),
                                stop=(cc == 1),
                            )
                    nc.scalar.copy(out=osb[:, :, ia:ib], in_=po2[:, :, ia:ib])
                else:
                    igroups = [(a, b) for (a, b) in ((ia, min(ib, 4)), (max(ia, 4), ib)) if a < b]
                    for o in range(2):
                        po = ps_o.tile([128, L, 128], f32, tag="pso")
                        for (i0, i1) in igroups:
                            for cc in range(2):
                                nc.tensor.matmul(
                                    out=po[:, i0:i1, :P],
                                    lhsT=wout_sb[:, cc, o * 128:(o + 1) * 128],
                                    rhs=aoti[:, i0:i1, cc, :P],
                                    start=(cc == 0),
                                    stop=(cc == 1),
                                )
                        nc.scalar.copy(out=osb[:, o, ia:ib], in_=po[:, ia:ib])
                if not do_dma:
                    return
                if st["rot"]:
                    (p0, b, h, w0, w1) = st["pieces"][0]
                    n = w1 - w0
                    for o in range(2):
                        # lower partition half: slots ia:ib are real l ia:ib
                        nc.sync.dma_start(
                            out=outT_d[o, :, b, h, ia:ib, w0:w1],
                            in_=osb[:, o, ia:ib, 0:n],
                        )
                        # upper half: slot s (s < 2) is real l s+3; slot 2 is a
                        # discarded duplicate
                        ja, jb = ia, min(ib, 2)
                        if ja < jb:
                            nc.sync.dma_start(
                                out=outT_d[o, :, b, h, ja + 3:jb + 3, w0:w1],
                                in_=osb[:, o, ja:jb, n:2 * n],
                            )
                else:
                    for (p0, b, h, w0, w1) in st["pieces"]:
                        for o in range(2):
                            nc.sync.dma_start(
                                out=outT_d[o, :, b, h, ia:ib, w0:w1],
                                in_=osb[:, o, ia:ib, p0:p0 + (w1 - w0)],
                            )

            # 3-deep software pipeline: per iteration t issue
            #   load(t)+proj(t)    DMA + PE qkv + ACT cast
            #   stage_a1(t-1)      DVE qk -> ACT exp
            #   stage_b(t-2)       DVE av (covers the exp round-trip)
            #   stage_a2(t-1)      DVE softmax tail
            #   stage_b_out(t-2)   PE transpose + out-proj; ACT cast; DMA store
            sts = {}
            for t in range(NTILES + 2):
                if t < NTILES:
                    sts[t] = load(t)
                    proj(sts[t])
                    if t == 0:
                        setup_tail()
                if 0 <= t - 1 < NTILES:
                    stage_a1(sts[t - 1])
                if 0 <= t - 2 < NTILES:
                    if t - 2 == NTILES - 1:
                        # last tile: i-extent 3 (rotated duplicate layout) and
                        # per-i backend chunks so its PE/ACT/DMA tail overlaps
                        # the tail of the DVE work; the output DMAs are
                        # coarsened into one final batch (10 small late DMAs
                        # serialized ~3us of drain otherwise)
                        for i in range(3):
                            stage_b_av(sts[t - 2], i, i + 1)
                            stage_b_out(sts[t - 2], i, i + 1, do_dma=False,
                                        do_proj=False)
                        stage_b_out(sts[t - 2], 0, 3, do_transpose=False)
                    else:
                        stage_b_av(sts[t - 2], 0, 5)
                if 0 <= t - 1 < NTILES:
                    stage_a2(sts[t - 1])
                if 0 <= t - 2 < NTILES:
                    if t - 2 != NTILES - 1:
                        stage_b_out(sts[t - 2], 0, 5)
                    del sts[t - 2]
    nc.finalize()  # Bacc.compile(): legalize multi-wait instructions, alloc regs
    return nc


def get_nc():
    if "nc" not in _cached:
        _cached["nc"] = _build_bass()
    return _cached["nc"]


def make_in_maps(x, mask, w_qkv, w_out, b_out):
    """Host-side shard + repack: x is transposed to [cc, c, b, h, l, w] and
    cast to bf16; the mask becomes an f32 additive logit bias."""
    import ml_dtypes

    x = np.asarray(x, dtype=np.float32)
    mask = np.asarray(mask)
    w_qkv = np.ascontiguousarray(np.asarray(w_qkv), dtype=np.float32)
    w_out = np.ascontiguousarray(np.asarray(w_out), dtype=np.float32)

    # permute V's output columns (m,d)->(d,m) and w_out's rows to match, so
    # the device-side pw broadcast is never on the innermost dim
    wv = w_qkv[:, 2 * INNER:].reshape(C, HEADS, DIM_HEAD).transpose(0, 2, 1).reshape(C, INNER)
    w_qkv = np.ascontiguousarray(
        np.concatenate([w_qkv[:, :2 * INNER], wv], axis=1).astype(ml_dtypes.bfloat16)
    )
    w_out = np.ascontiguousarray(
        w_out.reshape(HEADS, DIM_HEAD, C).transpose(1, 0, 2).reshape(INNER, C)
        .astype(ml_dtypes.bfloat16)
    )

    # [B, L, H, W, C] -> [C, B, H, L, W] -> [2, 128, B, H, L, W] bf16
    xT = np.transpose(x, (4, 0, 2, 1, 3)).astype(ml_dtypes.bfloat16)
    xT = np.ascontiguousarray(xT.reshape(2, 128, B, H, L, W))
    # [B, H, W, 1, L] -> bf16 multiplicative mask [B, H, W, L]
    mb = np.ascontiguousarray(
        (mask[:, :, :, 0, :] != 0).astype(ml_dtypes.bfloat16)
    )

    in_maps = []
    for k in range(NCORES):
        h0, h1 = k * HP, (k + 1) * HP
        in_maps.append({
            "xT": np.ascontiguousarray(xT[:, :, :, h0:h1]),
            "mbias": np.ascontiguousarray(mb[:, h0:h1]),
            "w_qkv": w_qkv,
            "w_out": w_out,
        })
    return in_maps


def assemble_out(results, b_out):
    """Host-side unshard: out^T bf16 [2, 128, B, HP, L, W] per core ->
    full f32 [B, L, H, W, C] (+ b_out)."""
    outT = np.concatenate([r["outT"] for r in results], axis=3)  # [2,128,B,H,L,W]
    out = np.transpose(outT, (2, 4, 3, 5, 0, 1)).reshape(B, L, H, W, C)
    return out.astype(np.float32) + np.asarray(b_out, dtype=np.float32)


def kernel(x, mask, w_qkv, w_out, b_out):
    from concourse.bass_utils import run_bass_kernel_spmd

    nc = get_nc()
    in_maps = make_in_maps(x, mask, w_qkv, w_out, b_out)
    res = run_bass_kernel_spmd(nc, in_maps, core_ids=list(range(NCORES)))
    return assemble_out(res.results, b_out)
